# revision 1
# baseline (speedup 1.0000x reference)
"""Trainium2 Bass kernel for nn_ConstellationRelay.

Computation (per token, D=1024, A=16 anchors, C=8 comps, dc=64):
  h   = l2norm(layernorm(x; ln_g, ln_b))
  tri = 1 - h @ l2norm(anchors).T                       (N, 16)
  u   = relu(einsum('nak,kae->nke', tri_g, W1) + b1)^2  (N, 8, 128)
  y   = layernorm_c(u @ W2 + b2; cg, cb)                (N, 8, 64)
  out = x + sigmoid(gate) * (y.flat @ Wp + bp)

Strategy: pure data-parallel over batch (one of 8 NeuronCores per batch row).
On-device fast path requires ln_g==1, ln_b==0 (always true for this problem's
setup_inputs); every other parameter is handled generally via host-side
folding:
  * h = (x - mu)/sqrt(1024*var)  -- eps cancels exactly through the l2norm
  * tri/W1 stage folded into two small matmuls; biasu (sum_m W1exp + b1) is
    folded into the expand matmul via a constant-1 row in a0, so squared-ReLU
    is a single (max 0) * self op
  * comp-LN mean-subtraction folded into centered W2/b2 (host)
  * cg, cb, bp, sigmoid(gate) folded into Wp/const (host); Wp is fp8 with a
    power-of-2 scale SP compensated in the residual add
  * proj and variance matmuls run in fp8 DoubleRow mode (2 k-chunks/instr)
Layout: token-major for stats/residual, feature-major (via DMA-transpose of
bf16 h) for all matmuls; proj matmul operand-swapped so the residual add
lands token-major in PSUM.
"""

import functools
import os
import sys

import numpy as np

for _p in ("/opt/trn_rl_repo",):
    if _p not in sys.path and os.path.isdir(_p):
        sys.path.insert(0, _p)

B, S, D = 8, 4096, 1024
A, C, DC = 16, 8, 64
APC = A // C  # anchors per compartment
E2 = 2 * DC  # 128, expanded width per comp
NCORES = 8
TOK = 512  # tokens per pipeline tile
NTILE = S // TOK  # 8
NCH = TOK // 128  # 4 token chunks of 128 per tile
KD = D // 128  # 8 feature chunks
SP = 256.0  # fp8 scale on the folded projection matrix


def _np_reference(x, anchors, ln_g, ln_b, W1, b1, W2, b2, cg, cb, Wp, bp, gate):
    """Pure-numpy fallback, mirrors reference.py (used only if ln_g/ln_b
    deviate from the values this problem's setup_inputs produces)."""
    x = x.astype(np.float32)
    N = x.shape[0] * x.shape[1]
    xf = x.reshape(N, D)
    mu = xf.mean(-1, keepdims=True)
    var = ((xf - mu) ** 2).mean(-1, keepdims=True)
    h = (xf - mu) / np.sqrt(var + 1e-5) * ln_g + ln_b
    h = h / np.maximum(np.linalg.norm(h, axis=-1, keepdims=True), 1e-12)
    a = anchors / np.maximum(np.linalg.norm(anchors, axis=-1, keepdims=True), 1e-12)
    tri = 1.0 - h @ a.T
    g = tri.reshape(N, APC, C)
    u = np.einsum("nak,kae->nke", g, W1) + b1
    u = np.square(np.maximum(u, 0.0))
    y = np.einsum("nke,ked->nkd", u, W2) + b2
    muy = y.mean(-1, keepdims=True)
    vy = ((y - muy) ** 2).mean(-1, keepdims=True)
    y = (y - muy) / np.sqrt(vy + 1e-5) * cg + cb
    upd = y.reshape(N, C * DC) @ Wp + bp
    sig = 1.0 / (1.0 + np.exp(-gate))
    return (xf + sig * upd).reshape(x.shape).astype(np.float32)


@functools.lru_cache(maxsize=4)
def _build_program(n_tokens=S, use_const=False):
    """Build + schedule the single-core Bass program (same program runs SPMD
    on all 8 cores)."""
    import concourse.bacc as bacc
    import concourse.mybir as mybir
    import concourse.tile as tile

    f32 = mybir.dt.float32
    bf16 = mybir.dt.bfloat16
    fp8 = mybir.dt.float8e4
    AF = mybir.ActivationFunctionType
    OP = mybir.AluOpType
    DR = mybir.MatmulPerfMode.DoubleRow

    ntile = n_tokens // TOK

    nc = bacc.Bacc("TRN2", target_bir_lowering=False, debug=False,
                   num_devices=NCORES)

    x_d = nc.dram_tensor("x", [n_tokens, D], f32, kind="ExternalInput")
    agt_d = nc.dram_tensor("agt", [128, 4, 2, 128], fp8, kind="ExternalInput")
    w1e_d = nc.dram_tensor("w1e", [128, KD, 128], bf16, kind="ExternalInput")
    w2c_d = nc.dram_tensor("w2c", [128, C, DC], bf16, kind="ExternalInput")
    vstl_d = nc.dram_tensor("vstl", [128, 4, C], bf16, kind="ExternalInput")
    b2f_d = nc.dram_tensor("b2f", [128, 4], f32, kind="ExternalInput")
    wpf_d = nc.dram_tensor("wpf", [128, 2, 2, 2, 512], fp8,
                           kind="ExternalInput")
    sel_d = nc.dram_tensor("sel", [C, 4, 128], bf16, kind="ExternalInput")
    cvec_d = nc.dram_tensor("cvec", [1, 2, 512], bf16, kind="ExternalInput") \
        if use_const else None
    out_d = nc.dram_tensor("out", [n_tokens, D], f32, kind="ExternalOutput")

    from contextlib import ExitStack

    with tile.TileContext(nc) as tc, ExitStack() as ctx:
        ctx.enter_context(nc.allow_low_precision(
            reason="update path is damped by sigmoid(gate)~0.047; fp8/bf16 "
                   "intermediates are well within the 2e-2 tolerance"))
        pp = ctx.enter_context(tc.tile_pool(name="params", bufs=1))
        agt = pp.tile([128, 4, 2, 128], fp8)
        nc.sync.dma_start(out=agt, in_=agt_d[:, :, :, :])
        w1e = pp.tile([128, KD, 128], bf16)
        nc.sync.dma_start(out=w1e, in_=w1e_d[:, :, :])
        w2c = pp.tile([128, C, DC], bf16)
        nc.sync.dma_start(out=w2c, in_=w2c_d[:, :, :])
        vstl = pp.tile([128, 4, C], bf16)
        nc.sync.dma_start(out=vstl, in_=vstl_d[:, :, :])
        b2f = pp.tile([128, 4], f32)
        nc.sync.dma_start(out=b2f, in_=b2f_d[:, :])
        wpf = pp.tile([128, 2, 2, 2, 512], fp8)
        nc.sync.dma_start(out=wpf, in_=wpf_d[:, :, :, :, :])
        sel = pp.tile([C, 4, 128], bf16)
        nc.sync.dma_start(out=sel, in_=sel_d[:, :, :])
        if use_const:
            cvec = pp.tile([1, 2, 512], bf16)
            nc.sync.dma_start(out=cvec, in_=cvec_d[:, :, :])
            ones1 = pp.tile([1, 128], bf16)
            nc.vector.memset(ones1, 1.0)
        ctiny = pp.tile([128, 1], f32)
        nc.vector.memset(ctiny, 1e-38)
        cepsp = pp.tile([C, 1], f32)
        nc.vector.memset(cepsp, 1e-5)
        # Constant-1 routing for biasu: a0p[32r+16, :] = 1.0 via a rank-1
        # accumulation appended to the A0 matmul.
        ones512 = pp.tile([1, TOK], bf16)
        nc.vector.memset(ones512, 1.0)
        bsel = pp.tile([1, 128], bf16)
        nc.vector.memset(bsel, 0.0)
        for r in range(4):
            nc.vector.memset(bsel[0:1, 32 * r + A:32 * r + A + 1], 1.0)

        px = ctx.enter_context(tc.tile_pool(name="px", bufs=2))
        psm = ctx.enter_context(tc.tile_pool(name="psm", bufs=8))
        # PSUM pools: 3 + 2 + 2 + 1 = 8 banks exactly.
        ps_exp = ctx.enter_context(tc.tile_pool(name="ps_exp", bufs=3,
                                                space="PSUM"))
        ps_y = ctx.enter_context(tc.tile_pool(name="ps_y", bufs=2,
                                              space="PSUM"))
        ps_mm = ctx.enter_context(tc.tile_pool(name="ps_mm", bufs=2,
                                               space="PSUM"))
        ps_small = ctx.enter_context(tc.tile_pool(name="ps_small", bufs=1,
                                                  space="PSUM"))

        def stage_front_load(t):
            row0 = t * TOK
            xt = px.tile([128, NCH, D], f32, tag="xt", bufs=4, name=f"xt{t}")
            nc.sync.dma_start(
                out=xt,
                in_=x_d[row0: row0 + TOK, :].rearrange(
                    "(c p) d -> p c d", p=128))
            return xt

        def stage_front_stats(t, xt):
            """Stats + normalize + transpose."""
            hb = px.tile([128, NCH, 512], bf16, tag="hb", bufs=2,
                         name=f"hb{t}")
            mv = psm.tile([128, NCH, 2], f32, tag="mv", name=f"mv{t}")
            for cch in range(NCH):
                st = psm.tile([128, 2, 6], f32, tag="st")
                xr = xt[:, cch, :].rearrange("p (s f) -> p s f", s=2)
                nc.vector.bn_stats(out=st[:, 0, :], in_=xr[:, 0, :])
                nc.vector.bn_stats(out=st[:, 1, :], in_=xr[:, 1, :])
                nc.vector.bn_aggr(out=mv[:, cch, :], in_=st)
            # ee = 16/sqrt(D*var + tiny) = 16/||x-mu|| (fp8 h scaled by 16)
            sd = psm.tile([128, NCH], f32, tag="sd")
            nc.scalar.activation(sd, mv[:, :, 1], AF.Sqrt, bias=ctiny,
                                 scale=float(D) / 256.0)
            ee = psm.tile([128, NCH], f32, tag="ee", name=f"ee{t}")
            nc.vector.reciprocal(ee, sd)
            bh = psm.tile([128, NCH], f32, tag="bh", name=f"bh{t}")
            nc.vector.scalar_tensor_tensor(
                out=bh, in0=mv[:, :, 0], scalar=-1.0, in1=ee,
                op0=OP.mult, op1=OP.mult)
            nmu = psm.tile([128, NCH], f32, tag="nmu", name=f"nmu{t}")
            nc.vector.tensor_scalar_mul(nmu, mv[:, :, 0], -1.0)
            # hb word w packs fp8 pair (16*h[w], 16*h[w+512]); the Act/Pool
            # out AP iterates the pair halves as the outer free dim
            # (chunk 0 on Act, chunks 1-3 on Pool)
            hw0 = hb[:, 0, :].bitcast(fp8).rearrange("p (w i) -> p i w", i=2)
            nc.scalar.activation(hw0, xt[:, 0, :], AF.Identity,
                                 bias=bh[:, 0:1], scale=ee[:, 0:1])
            for cch in range(1, NCH):
                hwc = hb[:, cch, :].bitcast(fp8).rearrange(
                    "p (w i) -> p i w", i=2)
                nc.gpsimd.tensor_scalar(
                    out=hwc, in0=xt[:, cch, :],
                    scalar1=nmu[:, cch:cch + 1], scalar2=ee[:, cch:cch + 1],
                    op0=OP.add, op1=OP.mult)
            hbT = px.tile([128, 4, TOK], bf16, tag="hbT", bufs=2,
                          name=f"hbT{t}")
            for cch in range(NCH):
                nc.sync.dma_start_transpose(
                    out=hbT[:, :, cch * 128:(cch + 1) * 128],
                    in_=hb[:, cch, :])
            return hbT

        def stage_mid_a0(t, xt, hbT):
            # --- A0 = a_norm @ h, 4 replicas at partitions {0,32,64,96};
            #     rows 32r+16 get the constant 1.0 that routes biasu through
            #     the expand matmul (rank-1 accumulation) -------------------
            a0p = ps_small.tile([128, TOK], f32, tag="small")
            for s in range(4):
                rhs8 = hbT[:, s, :].bitcast(fp8).rearrange(
                    "p (n i) -> p i n", i=2)
                nc.tensor.matmul(a0p, lhsT=agt[:, s, :, :], rhs=rhs8,
                                 start=(s == 0), stop=False, perf_mode=DR)
            nc.tensor.matmul(a0p, lhsT=bsel, rhs=ones512,
                             start=False, stop=True)
            a0 = psm.tile([128, TOK], bf16, tag="a0", bufs=2)
            nc.scalar.copy(out=a0, in_=a0p)
            return a0

        def stage_mid(t, xt, hbT, a0):
            # --- expand (4-way row-packed, biasu folded via const row);
            #     relu lands bf16 in SBUF, square runs on the DVE 2x path ----
            rb = px.tile([128, KD, TOK], bf16, tag="rb", bufs=2)
            ubig = px.tile([128, KD, TOK], bf16, tag="ubig", bufs=2)
            for kg in range(2):
                ups = []
                for r in range(4):
                    k = 4 * kg + r
                    up = ps_exp.tile([128, TOK], f32, tag="exp")
                    nc.tensor.matmul(
                        up, lhsT=w1e[32 * r:32 * r + A + 1, k, :],
                        rhs=a0[32 * r:32 * r + A + 1, :],
                        start=True, stop=True,
                        tile_position=(32 * r, 0))
                    ups.append(up)
                for r in range(4):
                    k = 4 * kg + r
                    nc.scalar.activation(rb[:, k, :], ups[r], AF.Relu)
                    nc.gpsimd.tensor_mul(ubig[:, k, :], rb[:, k, :],
                                         rb[:, k, :])

            # --- comp matmul; yb (biased, fp8) + sqy = (yp+b2f)^2 ----------
            yb = px.tile([128, 4, TOK], fp8, tag="yb", bufs=4, name=f"yb{t}")
            sqy = px.tile([128, 4, TOK], bf16, tag="sqy", bufs=3, name=f"sqy{t}")
            for j in range(4):
                yp = ps_y.tile([128, TOK], f32, tag="ypre")
                nc.tensor.matmul(yp[0:64, :], lhsT=w2c[:, 2 * j, :],
                                 rhs=ubig[:, 2 * j, :], start=True, stop=True)
                nc.tensor.matmul(yp[64:128, :], lhsT=w2c[:, 2 * j + 1, :],
                                 rhs=ubig[:, 2 * j + 1, :], start=True,
                                 stop=True, tile_position=(0, 64))
                nc.scalar.activation(yb[:, j, :], yp, AF.Identity,
                                     bias=b2f[:, j:j + 1], scale=1.0)
                nc.scalar.activation(sqy[:, j, :], yp, AF.Square,
                                     bias=b2f[:, j:j + 1], scale=1.0)

            return xt, yb, sqy

        def stage_var(t, sqy):
            # --- per-comp variance matmul; rstd = 1/sqrt(var+eps) ----------
            vst = ps_small.tile([C, TOK], f32, tag="small")
            for j in range(4):
                nc.tensor.matmul(vst, lhsT=vstl[:, j, :], rhs=sqy[:, j, :],
                                 start=(j == 0), stop=(j == 3))
            sd2 = psm.tile([C, TOK], f32, tag="sd2", bufs=2)
            nc.scalar.activation(sd2, vst, AF.Sqrt, bias=cepsp, scale=1.0)
            rr = psm.tile([C, TOK], f32, tag="rr", bufs=2)
            nc.vector.reciprocal_approx_fast(out=rr, in_=sd2)
            rrb = psm.tile([C, TOK], bf16, tag="rrb", bufs=3, name=f"rrb{t}")
            nc.vector.tensor_copy(out=rrb, in_=rr)
            return rrb

        def stage_back(t, xt, yb, rrb):
            row0 = t * TOK
            # rstd broadcast via selector matmuls; ycT = yb * rstd (fp8)
            ycT = px.tile([128, 4, TOK], fp8, tag="ycT", bufs=2)
            for j in range(4):
                rbP = ps_mm.tile([128, TOK], f32, tag="mmout")
                nc.tensor.matmul(rbP, lhsT=sel[:, j, :], rhs=rrb,
                                 start=True, stop=True)
                nc.vector.tensor_mul(ycT[:, j, :], yb[:, j, :], rbP)

            # --- proj (fp8 DoubleRow, operand-swapped) + residual ----------
            upd = px.tile([128, NCH, 2, 512], bf16, tag="upd", bufs=1,
                          name=f"upd{t}")
            for cch in range(NCH):
                osb = px.tile([128, D], f32, tag="osb", bufs=3,
                              name=f"osb{t}_{cch}")
                ud = [ps_mm.tile([128, 512], f32, tag="mmout",
                                 name=f"ud{t}_{cch}_{i}") for i in range(2)]
                for m in range(2):
                    for hf in range(2):
                        nc.tensor.matmul(
                            ud[hf],
                            lhsT=ycT[:, 2 * m:2 * m + 2,
                                     cch * 128:(cch + 1) * 128],
                            rhs=wpf[:, m, :, hf, :],
                            start=(m == 0),
                            stop=(m == 1 and not use_const),
                            perf_mode=DR)
                if use_const:
                    for hf in range(2):
                        nc.tensor.matmul(ud[hf], lhsT=ones1,
                                         rhs=cvec[:, hf, :],
                                         start=False, stop=True)
                for hf in range(2):
                    idx = cch * 2 + hf
                    dst = osb[:, hf * 512:(hf + 1) * 512]
                    xs = xt[:, cch, hf * 512:(hf + 1) * 512]
                    if idx in (1, 3, 4, 6):
                        # Act materializes update (bf16), Pool adds residual
                        nc.scalar.activation(upd[:, cch, hf, :], ud[hf],
                                             AF.Identity, scale=1.0 / SP)
                        nc.gpsimd.tensor_tensor(
                            out=dst, in0=upd[:, cch, hf, :], in1=xs,
                            op=OP.add)
                    else:
                        nc.vector.scalar_tensor_tensor(
                            out=dst, in0=ud[hf], scalar=1.0 / SP,
                            in1=xs, op0=OP.mult, op1=OP.add)
                nc.sync.dma_start(
                    out=out_d[row0 + cch * 128: row0 + (cch + 1) * 128, :],
                    in_=osb)

        ld = {}
        fr = {}
        md = {}
        vr = {}
        for t in range(ntile + 3):
            if t < ntile:
                ld[t] = stage_front_load(t)
            if 2 <= t <= ntile + 1:
                xtm, ybm, sqym = md.pop(t - 2)
                rrb_ = stage_var(t - 2, sqym)
                vr[t - 2] = (xtm, ybm, rrb_)
            if 1 <= t <= ntile:
                xt_, hbT_ = fr.pop(t - 1)
                a0_ = stage_mid_a0(t - 1, xt_, hbT_)
            if t >= 3:
                xtb, ybb, rrbb = vr.pop(t - 3)
                stage_back(t - 3, xtb, ybb, rrbb)
            if t < ntile:
                xtf = ld.pop(t)
                fr[t] = (xtf, stage_front_stats(t, xtf))
            if 1 <= t <= ntile:
                md[t - 1] = stage_mid(t - 1, xt_, hbT_, a0_)

    nc.compile()
    return nc


def _pack_params(anchors, ln_g, W1, b1, W2, b2, cg, cb, Wp, bp, gate):
    import ml_dtypes
    f32 = np.float32
    bf16 = ml_dtypes.bfloat16
    fp8 = ml_dtypes.float8_e4m3

    anchors = anchors.astype(f32)
    an = anchors / np.maximum(
        np.linalg.norm(anchors.astype(np.float64), axis=1, keepdims=True),
        1e-12).astype(f32)
    ag = (an * ln_g[None, :].astype(f32)).astype(f32)  # [A, D]

    # agt[p, s, i, 32r+m] = 16*ag[m, 4p+s+512i] for r in 0..3 (4 replicas);
    # transposed 16-bit word w=(4p+s) holds the fp8 pair (h[w], h[w+512])
    agt = np.zeros((128, 4, 2, 128), f32)
    ww = np.arange(512)
    pidx, sidx = ww // 4, ww % 4
    for i in range(2):
        for r in range(4):
            agt[pidx, sidx, i, 32 * r:32 * r + A] = 16.0 * ag.T[ww + 512 * i]

    # W1exp[m, f] with m=j*C+k2, f=k*128+e -> value W1[k, j, e] iff k2==k
    W1 = W1.astype(f32)
    w1exp = np.zeros((A, C, E2), f32)
    for m in range(A):
        j, k2 = m // C, m % C
        w1exp[m, k2, :] = W1[k2, j, :]
    sf = w1exp.sum(axis=0)  # [C, E2]
    biasu = sf + b1.astype(f32)  # [C, E2]
    # w1e rows 0..15 = -W1exp (replicated 4x); row 16 = biasu (rides the
    # constant-1 row of a0)
    # anchor rows see a0 = 256*(h.a); bias row rides the exact 1.0 const
    w1e = np.zeros((128, C, E2), f32)
    for r in range(4):
        w1e[32 * r:32 * r + A] = -w1exp / 256.0
        w1e[32 * r + A] = biasu

    W2 = W2.astype(f32)
    w2m = W2.mean(axis=2, keepdims=True)
    w2cent = W2 - w2m  # [C, E2, DC]
    w2c = np.transpose(w2cent, (1, 0, 2)).copy()  # [128, C, 64]
    b2c = b2.astype(f32) - b2.astype(f32).mean(axis=1, keepdims=True)  # [C, DC]

    b2f = np.zeros((128, 4), f32)
    for j in range(4):
        for p in range(128):
            kk = 2 * j + p // 64
            b2f[p, j] = b2c[kk, p % 64]

    # vstl[p, j, c] = 1/64 iff c == 2*j + p//64
    vstl = np.zeros((128, 4, C), f32)
    for j in range(4):
        for p in range(128):
            vstl[p, j, 2 * j + p // 64] = 1.0 / DC

    sig = (1.0 / (1.0 + np.exp(-gate.astype(np.float64)))).astype(f32)  # [D]
    wpfold = (cg.astype(f32).reshape(C * DC, 1) * Wp.astype(f32)) * sig[None, :]
    # wpf[p, m, i, hf, n] = SP * wpfold[(2m+i)*128+p, hf*512+n]
    wpf = np.ascontiguousarray(
        (SP * wpfold).reshape(2, 2, 128, 2, 512).transpose(2, 0, 1, 3, 4))

    const = (cb.astype(f32).reshape(-1) @ Wp.astype(f32) + bp.astype(f32)) * sig
    use_const = bool(np.max(np.abs(const)) > 0)

    sel = np.zeros((C, 4, 128), f32)
    for j in range(4):
        sel[2 * j, j, 0:64] = 1.0
        sel[2 * j + 1, j, 64:128] = 1.0

    params = dict(
        sel=sel.astype(bf16),
        agt=agt.astype(fp8),
        w1e=w1e.astype(bf16),
        w2c=w2c.astype(bf16),
        vstl=vstl.astype(bf16),
        b2f=b2f.astype(f32),
        wpf=wpf.astype(fp8),
    )
    if use_const:
        params["cvec"] = (SP * const).reshape(1, 2, 512).astype(bf16)
    return params, use_const


def kernel(**inputs):
    x = np.asarray(inputs["x"], dtype=np.float32)
    ln_g = np.asarray(inputs["ln_g"], dtype=np.float32)
    ln_b = np.asarray(inputs["ln_b"], dtype=np.float32)

    fast = (np.allclose(ln_g, 1.0, atol=1e-12) and
            np.allclose(ln_b, 0.0, atol=1e-12))
    if not fast:
        return _np_reference(
            x, *[np.asarray(inputs[k], dtype=np.float32) for k in
                 ("anchors", "ln_g", "ln_b", "W1", "b1", "W2", "b2", "cg",
                  "cb", "Wp", "bp", "gate")])

    params, use_const = _pack_params(
        inputs["anchors"], ln_g, inputs["W1"], inputs["b1"], inputs["W2"],
        inputs["b2"], inputs["cg"], inputs["cb"], inputs["Wp"], inputs["bp"],
        inputs["gate"])

    nc = _build_program(S, use_const)

    from concourse.bass_utils import run_bass_kernel_spmd
    in_maps = []
    for b in range(NCORES):
        m = dict(params)
        m["x"] = np.ascontiguousarray(x[b])
        in_maps.append(m)
    res = run_bass_kernel_spmd(nc, in_maps, core_ids=list(range(NCORES)))
    out = np.stack([res.results[b]["out"] for b in range(NCORES)], axis=0)
    return out.reshape(B, S, D).astype(np.float32)



# revision 10
# speedup vs baseline: 1.1955x; 1.1955x over previous
"""Trainium2 Bass kernel for nn_ConstellationRelay (v2).

Computation (per token, D=1024, A=16 anchors, C=8 comps, dc=64):
  h   = l2norm(layernorm(x; ln_g, ln_b))
  tri = 1 - h @ l2norm(anchors).T                       (N, 16)
  u   = relu(einsum('nak,kae->nke', tri_g, W1) + b1)^2  (N, 8, 128)
  y   = layernorm_c(u @ W2 + b2; cg, cb)                (N, 8, 64)
  out = x + sigmoid(gate) * (y.flat @ Wp + bp)

Strategy: pure data-parallel over batch (one NeuronCore per batch row).
Fast path requires ln_g==1, ln_b==0 (true for this problem); general
inputs fall back to numpy.

Key structure (vs v1):
  * x is uploaded TWICE: token-major bf16 (stats + residual) and
    feature-major fp8 pairs packed on host (a0 matmul rhs).  No on-device
    normalize pass or transpose: LN mean-centering is folded into the
    anchors (a_c = a_n - rowmean contribution), so h.a = (x.a_c)/||x-mu||.
  * ee = 4/||x-mu|| from bn_stats+Rsqrt; transposed via a tiny PE
    transpose and broadcast to [128,512] with 4 selector matmuls; a0 is
    scaled by it in the PSUM->SBUF copy (DVE).
  * biasu is applied via the relu: on Act relu(-up+biasu), on Pool
    min(up-biasu,0) (sign dies in the square).
  * squared-relu / sqy are bf16 SBUF->SBUF DVE ops (2x/4x perf modes).
  * yb (bf16) frees the comp PSUM early; ycT = yb * rstd_bcast -> fp8
    feeds the fp8 DoubleRow projection; residual adds are single
    scalar_tensor_tensor ops (ud/SP + x) split across DVE/Pool.
  * everything bf16/fp8 on the wire: in 8.4+4.2 MB, out 8.4 MB per core.
  * Only Rsqrt/Relu/Square/Identity/Copy activations -> one act table.
"""

import functools
import os
import sys

import numpy as np

for _p in ("/opt/trn_rl_repo",):
    if _p not in sys.path and os.path.isdir(_p):
        sys.path.insert(0, _p)

B, S, D = 8, 4096, 1024
A, C, DC = 16, 8, 64
APC = A // C
E2 = 2 * DC  # 128
NCORES = 8
TOK = 512
NTILE = S // TOK  # 8
NCH = TOK // 128  # 4
KD = D // 128  # 8
SP = 256.0  # fp8 scale on the folded projection matrix

# engine split knobs.  NOTE: GpSimd (Pool) cannot read PSUM, so every
# PSUM-consuming op runs on Act (activation-class) or DVE; Pool only gets
# SBUF->SBUF work (sqy square, residual adds fed by an Act copy).
RELU_DVE = (1, 4)              # ks whose squared-relu runs on DVE (rest Act)
RES_DVE_HF = 0                 # residual half on DVE-STT; other half Act+Pool


def _np_reference(x, anchors, ln_g, ln_b, W1, b1, W2, b2, cg, cb, Wp, bp, gate):
    x = x.astype(np.float32)
    N = x.shape[0] * x.shape[1]
    xf = x.reshape(N, D)
    mu = xf.mean(-1, keepdims=True)
    var = ((xf - mu) ** 2).mean(-1, keepdims=True)
    h = (xf - mu) / np.sqrt(var + 1e-5) * ln_g + ln_b
    h = h / np.maximum(np.linalg.norm(h, axis=-1, keepdims=True), 1e-12)
    a = anchors / np.maximum(np.linalg.norm(anchors, axis=-1, keepdims=True), 1e-12)
    tri = 1.0 - h @ a.T
    g = tri.reshape(N, APC, C)
    u = np.einsum("nak,kae->nke", g, W1) + b1
    u = np.square(np.maximum(u, 0.0))
    y = np.einsum("nke,ked->nkd", u, W2) + b2
    muy = y.mean(-1, keepdims=True)
    vy = ((y - muy) ** 2).mean(-1, keepdims=True)
    y = (y - muy) / np.sqrt(vy + 1e-5) * cg + cb
    upd = y.reshape(N, C * DC) @ Wp + bp
    sig = 1.0 / (1.0 + np.exp(-gate))
    return (xf + sig * upd).reshape(x.shape).astype(np.float32)


@functools.lru_cache(maxsize=4)
def _build_program(n_tokens=S, use_const=False):
    import concourse.bacc as bacc
    import concourse.mybir as mybir
    import concourse.tile as tile

    f32 = mybir.dt.float32
    bf16 = mybir.dt.bfloat16
    fp8 = mybir.dt.float8e4
    AF = mybir.ActivationFunctionType
    OP = mybir.AluOpType
    DR = mybir.MatmulPerfMode.DoubleRow

    ntile = n_tokens // TOK

    nc = bacc.Bacc("TRN2", target_bir_lowering=False, debug=False,
                   num_devices=NCORES)

    xb_d = nc.dram_tensor("xb", [n_tokens, D], bf16, kind="ExternalInput")
    xT_d = nc.dram_tensor("xT", [128, ntile, 4, TOK], bf16,
                          kind="ExternalInput")
    agt_d = nc.dram_tensor("agt", [128, 4, 2, 128], fp8, kind="ExternalInput")
    w1e_d = nc.dram_tensor("w1e", [128, KD, 128], bf16, kind="ExternalInput")
    biasu_d = nc.dram_tensor("biasu", [128, KD], f32, kind="ExternalInput")
    w2c_d = nc.dram_tensor("w2c", [128, C, DC], bf16, kind="ExternalInput")
    b2f_d = nc.dram_tensor("b2f", [128, 4], f32, kind="ExternalInput")
    vstl_d = nc.dram_tensor("vstl", [128, 4, C], bf16, kind="ExternalInput")
    sel_d = nc.dram_tensor("sel", [C, 4, 128], bf16, kind="ExternalInput")
    selc_d = nc.dram_tensor("selc", [4, 4, 128], bf16, kind="ExternalInput")
    ident_d = nc.dram_tensor("ident", [128, 128], bf16, kind="ExternalInput")
    wpf_d = nc.dram_tensor("wpf", [128, 2, 2, 2, 512], fp8,
                           kind="ExternalInput")
    cvec_d = nc.dram_tensor("cvec", [1, 2, 512], bf16, kind="ExternalInput") \
        if use_const else None
    out_d = nc.dram_tensor("out", [n_tokens, D], bf16, kind="ExternalOutput")

    from contextlib import ExitStack

    with tile.TileContext(nc) as tc, ExitStack() as ctx:
        ctx.enter_context(nc.allow_low_precision(
            reason="update path is damped by sigmoid(gate)~0.047; fp8/bf16 "
                   "intermediates are well within the 2e-2 tolerance"))
        pp = ctx.enter_context(tc.tile_pool(name="params", bufs=1))
        agt = pp.tile([128, 4, 2, 128], fp8)
        nc.sync.dma_start(out=agt, in_=agt_d[:, :, :, :])
        w1e = pp.tile([128, KD, 128], bf16)
        nc.sync.dma_start(out=w1e, in_=w1e_d[:, :, :])
        biasu = pp.tile([128, KD], f32)
        nc.sync.dma_start(out=biasu, in_=biasu_d[:, :])
        w2c = pp.tile([128, C, DC], bf16)
        nc.sync.dma_start(out=w2c, in_=w2c_d[:, :, :])
        b2f = pp.tile([128, 4], f32)
        nc.sync.dma_start(out=b2f, in_=b2f_d[:, :])
        vstl = pp.tile([128, 4, C], bf16)
        nc.sync.dma_start(out=vstl, in_=vstl_d[:, :, :])
        sel = pp.tile([C, 4, 128], bf16)
        nc.sync.dma_start(out=sel, in_=sel_d[:, :, :])
        selc = pp.tile([4, 4, 128], bf16)
        nc.sync.dma_start(out=selc, in_=selc_d[:, :, :])
        ident = pp.tile([128, 128], bf16)
        nc.sync.dma_start(out=ident, in_=ident_d[:, :])
        wpf = pp.tile([128, 2, 2, 2, 512], fp8)
        nc.sync.dma_start(out=wpf, in_=wpf_d[:, :, :, :, :])
        if use_const:
            cvec = pp.tile([1, 2, 512], bf16)
            nc.sync.dma_start(out=cvec, in_=cvec_d[:, :, :])
            ones1 = pp.tile([1, 128], bf16)
            nc.vector.memset(ones1, 1.0)
        ctiny = pp.tile([128, 1], f32)
        nc.vector.memset(ctiny, 1e-20)
        cepsp = pp.tile([C, 1], f32)
        nc.vector.memset(cepsp, 1e-5)

        px = ctx.enter_context(tc.tile_pool(name="px", bufs=2))
        psm = ctx.enter_context(tc.tile_pool(name="psm", bufs=2))
        # PSUM pools: 2 + 3 + 1 + 2 = 8 banks exactly.
        ps_y = ctx.enter_context(tc.tile_pool(name="ps_y", bufs=2,
                                              space="PSUM"))
        ps_exp = ctx.enter_context(tc.tile_pool(name="ps_exp", bufs=3,
                                                space="PSUM"))
        ps_r = ctx.enter_context(tc.tile_pool(name="ps_r", bufs=1,
                                              space="PSUM"))
        ps_u = ctx.enter_context(tc.tile_pool(name="ps_u", bufs=2,
                                              space="PSUM"))

        def stage_load(t):
            row0 = t * TOK
            xt = px.tile([128, NCH, D], bf16, tag="xt", bufs=5, name=f"xt{t}")
            nc.sync.dma_start(
                out=xt,
                in_=xb_d[row0: row0 + TOK, :].rearrange(
                    "(c p) d -> p c d", p=128))
            xT = px.tile([128, 4, TOK], bf16, tag="xT", bufs=3, name=f"xT{t}")
            nc.sync.dma_start(out=xT, in_=xT_d[:, t, :, :])
            return xt, xT

        def stage_stats_bn(t, xt):
            mv = psm.tile([128, NCH, 2], f32, tag="mv", name=f"mv{t}")
            for cch in range(NCH):
                st = psm.tile([128, 2, 6], f32, tag="st")
                xr = xt[:, cch, :].rearrange("p (s f) -> p s f", s=2)
                nc.vector.bn_stats(out=st[:, 0, :], in_=xr[:, 0, :])
                nc.vector.bn_stats(out=st[:, 1, :], in_=xr[:, 1, :])
                nc.vector.bn_aggr(out=mv[:, cch, :], in_=st)
            # ee = 4/||x-mu|| = 1/sqrt(D/16 * var)
            sd = psm.tile([128, NCH], f32, tag="sd")
            nc.scalar.activation(sd, mv[:, :, 1], AF.Sqrt, bias=ctiny,
                                 scale=float(D) / 16.0)
            eebf = psm.tile([128, NCH], bf16, tag="eebf", name=f"eebf{t}")
            nc.vector.reciprocal(eebf, sd)
            return eebf

        def stage_stats_tp(t, eebf):
            ee_t = ps_r.tile([NCH, 128], bf16, tag="r", name=f"eet{t}")
            nc.tensor.transpose(ee_t, eebf, ident)
            eeT = psm.tile([NCH, 128], bf16, tag="eeT", bufs=2,
                           name=f"eeT{t}")
            nc.scalar.copy(out=eeT, in_=ee_t)
            return eeT

        def stage_a0(t, xT, eeT):
            eeb = ps_y.tile([128, TOK], f32, tag="y", name=f"eeb{t}")
            for cch in range(NCH):
                nc.tensor.matmul(eeb[:, cch * 128:(cch + 1) * 128],
                                 lhsT=selc[:, cch, :], rhs=eeT,
                                 start=True, stop=True)
            eesb = px.tile([128, TOK], bf16, tag="eesb", bufs=2,
                           name=f"eesb{t}")
            nc.scalar.copy(out=eesb, in_=eeb)
            a0p = ps_y.tile([128, TOK], f32, tag="y", name=f"a0p{t}")
            for s in range(4):
                rhs8 = xT[:, s, :].bitcast(fp8).rearrange(
                    "p (n i) -> p i n", i=2)
                nc.tensor.matmul(a0p, lhsT=agt[:, s, :, :], rhs=rhs8,
                                 start=(s == 0), stop=(s == 3), perf_mode=DR)
            a0s = px.tile([128, TOK], bf16, tag="a0s", bufs=2, name=f"a0s{t}")
            nc.vector.tensor_mul(a0s, a0p, eesb)
            return a0s

        def stage_mid(t, a0s):
            # expand + squared relu
            rb = px.tile([128, KD, TOK], bf16, tag="rb", bufs=2)
            ubig = px.tile([128, KD, TOK], bf16, tag="ubig", bufs=2)
            for kg in range(2):
                ups = []
                for r in range(4):
                    k = 4 * kg + r
                    up = ps_exp.tile([128, TOK], f32, tag="exp")
                    nc.tensor.matmul(
                        up, lhsT=w1e[32 * r:32 * r + A, k, :],
                        rhs=a0s[32 * r:32 * r + A, :],
                        start=True, stop=True,
                        tile_position=(32 * r, 0))
                    ups.append(up)
                for r in range(4):
                    k = 4 * kg + r
                    if k in RELU_DVE:
                        # min(up-biasu, 0) = -relu(biasu-up); square kills sign
                        nc.vector.tensor_scalar(
                            out=rb[:, k, :], in0=ups[r],
                            scalar1=biasu[:, k:k + 1], scalar2=0.0,
                            op0=OP.subtract, op1=OP.min)
                    else:
                        nc.scalar.activation(rb[:, k, :], ups[r], AF.Relu,
                                             bias=biasu[:, k:k + 1],
                                             scale=-1.0)
                nc.vector.tensor_mul(ubig[:, 4 * kg:4 * kg + 4, :],
                                     rb[:, 4 * kg:4 * kg + 4, :],
                                     rb[:, 4 * kg:4 * kg + 4, :])

            # comp matmul; yb = yp + b2f (bf16, frees PSUM early)
            yb = px.tile([128, 4, TOK], bf16, tag="yb", bufs=3, name=f"yb{t}")
            for j in range(4):
                yp = ps_y.tile([128, TOK], f32, tag="y")
                nc.tensor.matmul(yp[0:64, :], lhsT=w2c[:, 2 * j, :],
                                 rhs=ubig[:, 2 * j, :], start=True, stop=True)
                nc.tensor.matmul(yp[64:128, :], lhsT=w2c[:, 2 * j + 1, :],
                                 rhs=ubig[:, 2 * j + 1, :], start=True,
                                 stop=True, tile_position=(0, 64))
                nc.scalar.activation(yb[:, j, :], yp, AF.Identity,
                                     bias=b2f[:, j:j + 1], scale=1.0)
            sqy = px.tile([128, 4, TOK], bf16, tag="sqy", bufs=2)
            nc.gpsimd.tensor_mul(sqy, yb, yb)

            # per-comp variance + rstd
            vst = ps_r.tile([C, TOK], f32, tag="r", name=f"vst{t}")
            for j in range(4):
                nc.tensor.matmul(vst, lhsT=vstl[:, j, :], rhs=sqy[:, j, :],
                                 start=(j == 0), stop=(j == 3))
            sd2 = psm.tile([C, TOK], f32, tag="sd2", bufs=2)
            nc.scalar.activation(sd2, vst, AF.Sqrt, bias=cepsp, scale=1.0)
            rrb = psm.tile([C, TOK], bf16, tag="rrb", bufs=2, name=f"rrb{t}")
            nc.vector.reciprocal(rrb, sd2)
            return yb, rrb

        def stage_back_sel(t, yb, rrb):
            ycT = px.tile([128, 4, TOK], fp8, tag="ycT", bufs=2,
                          name=f"ycT{t}")
            for j in range(4):
                rbP = ps_r.tile([128, TOK], f32, tag="r")
                nc.tensor.matmul(rbP, lhsT=sel[:, j, :], rhs=rrb,
                                 start=True, stop=True)
                nc.vector.tensor_mul(ycT[:, j, :], yb[:, j, :], rbP)
            return ycT

        def stage_back_proj(t, xt, ycT):
            row0 = t * TOK
            for cch in range(NCH):
                ud = [ps_u.tile([128, 512], f32, tag="ud",
                                name=f"ud{t}_{cch}_{i}") for i in range(2)]
                for m in range(2):
                    for hf in range(2):
                        nc.tensor.matmul(
                            ud[hf],
                            lhsT=ycT[:, 2 * m:2 * m + 2,
                                     cch * 128:(cch + 1) * 128],
                            rhs=wpf[:, m, :, hf, :],
                            start=(m == 0),
                            stop=(m == 1 and not use_const),
                            perf_mode=DR)
                if use_const:
                    for hf in range(2):
                        nc.tensor.matmul(ud[hf], lhsT=ones1,
                                         rhs=cvec[:, hf, :],
                                         start=False, stop=True)
                osb = px.tile([128, D], bf16, tag="osb", bufs=3,
                              name=f"osb{t}_{cch}")
                for hf in range(2):
                    xs = xt[:, cch, hf * 512:(hf + 1) * 512]
                    dst = osb[:, hf * 512:(hf + 1) * 512]
                    if hf == RES_DVE_HF:
                        nc.vector.scalar_tensor_tensor(
                            out=dst, in0=ud[hf], scalar=1.0 / SP,
                            in1=xs, op0=OP.mult, op1=OP.add)
                    else:
                        # Pool can't read PSUM: Act scales ud into SBUF,
                        # Pool adds the residual
                        uph = px.tile([128, 512], bf16, tag="uph", bufs=2,
                                      name=f"uph{t}_{cch}")
                        nc.scalar.activation(uph, ud[hf], AF.Identity,
                                             scale=1.0 / SP)
                        nc.gpsimd.tensor_tensor(out=dst, in0=uph, in1=xs,
                                                op=OP.add)
                nc.sync.dma_start(
                    out=out_d[row0 + cch * 128: row0 + (cch + 1) * 128, :],
                    in_=osb)

        ld = {}
        ee = {}
        a0 = {}
        md = {}
        yc = {}
        for s in range(ntile + 4):
            if s < ntile:
                ld[s] = stage_load(s)
            if 2 <= s <= ntile + 1:
                t = s - 2
                a0[t] = stage_a0(t, ld[t][1], ee.pop(t)[1])
            if 4 <= s <= ntile + 3:
                t = s - 4
                ybb, rrbb = md.pop(t)
                yc[t] = stage_back_sel(t, ybb, rrbb)
            if 1 <= s <= ntile:
                t = s - 1
                eebf_ = stage_stats_bn(t, ld[t][0])
            if 3 <= s <= ntile + 2:
                t = s - 3
                md[t] = stage_mid(t, a0.pop(t))
            if 4 <= s <= ntile + 3:
                t = s - 4
                stage_back_proj(t, ld.pop(t)[0], yc.pop(t))
            if 1 <= s <= ntile:
                t = s - 1
                ee[t] = (eebf_, stage_stats_tp(t, eebf_))

    nc.compile()
    return nc


def _pack_params(anchors, ln_g, W1, b1, W2, b2, cg, cb, Wp, bp, gate):
    import ml_dtypes
    f32 = np.float32
    bf16 = ml_dtypes.bfloat16
    fp8 = ml_dtypes.float8_e4m3

    anchors = anchors.astype(f32)
    an = anchors / np.maximum(
        np.linalg.norm(anchors.astype(np.float64), axis=1, keepdims=True),
        1e-12).astype(f32)
    ag = an * ln_g[None, :].astype(f32)  # [A, D]
    a_c = ag - ag.mean(axis=1, keepdims=True)  # fold LN mean-centering

    # agt[p, s, i, 32r+m] = 2*a_c[m, 4p+s+512i], 4 row replicas
    agt = np.zeros((128, 4, 2, 128), f32)
    ww = np.arange(512)
    pidx, sidx = ww // 4, ww % 4
    for i in range(2):
        for r in range(4):
            agt[pidx, sidx, i, 32 * r:32 * r + A] = 2.0 * a_c.T[ww + 512 * i]

    # W1exp[m, k, e] = W1[k, j, e] iff m == j*C + k
    W1 = W1.astype(f32)
    w1exp = np.zeros((A, C, E2), f32)
    for m in range(A):
        j, k2 = m // C, m % C
        w1exp[m, k2, :] = W1[k2, j, :]
    biasu_np = w1exp.sum(axis=0) + b1.astype(f32)  # [C, E2]
    # w1e rows 32r..32r+15 = +W1exp for comp k (r = k % 4)
    w1e = np.zeros((128, KD, 128), f32)
    for k in range(KD):
        r = k % 4
        w1e[32 * r:32 * r + A, k, :] = w1exp[:, k, :]
    biasu_sb = np.ascontiguousarray(biasu_np.T)  # [128, KD]

    W2 = W2.astype(f32)
    w2cent = W2 - W2.mean(axis=2, keepdims=True)
    w2c = np.transpose(w2cent, (1, 0, 2)).copy()  # [128, C, 64]
    b2c = b2.astype(f32) - b2.astype(f32).mean(axis=1, keepdims=True)

    b2f = np.zeros((128, 4), f32)
    for j in range(4):
        for p in range(128):
            kk = 2 * j + p // 64
            b2f[p, j] = b2c[kk, p % 64]

    # vstl[p, j, c] = 1/64 iff c == 2*j + p//64
    vstl = np.zeros((128, 4, C), f32)
    for j in range(4):
        for p in range(128):
            vstl[p, j, 2 * j + p // 64] = 1.0 / DC

    sig = (1.0 / (1.0 + np.exp(-gate.astype(np.float64)))).astype(f32)
    wpfold = (cg.astype(f32).reshape(C * DC, 1) * Wp.astype(f32)) * sig[None, :]
    wpf = np.ascontiguousarray(
        (SP * wpfold).reshape(2, 2, 128, 2, 512).transpose(2, 0, 1, 3, 4))

    const = (cb.astype(f32).reshape(-1) @ Wp.astype(f32) + bp.astype(f32)) * sig
    use_const = bool(np.max(np.abs(const)) > 0)

    sel_np = np.zeros((C, 4, 128), f32)
    for j in range(4):
        sel_np[2 * j, j, 0:64] = 1.0
        sel_np[2 * j + 1, j, 64:128] = 1.0

    selc = np.zeros((4, 4, 128), f32)
    for c in range(4):
        selc[c, c, :] = 1.0

    params = dict(
        agt=agt.astype(fp8),
        w1e=w1e.astype(bf16),
        biasu=biasu_sb.astype(f32),
        w2c=w2c.astype(bf16),
        b2f=b2f.astype(f32),
        vstl=vstl.astype(bf16),
        sel=sel_np.astype(bf16),
        selc=selc.astype(bf16),
        ident=np.eye(128, dtype=f32).astype(bf16),
        wpf=wpf.astype(fp8),
    )
    if use_const:
        params["cvec"] = (SP * const).reshape(1, 2, 512).astype(bf16)
    return params, use_const


def _pack_x(xcore):
    """Pack one core's x [S, D] f32 into (token-major bf16, feature-major
    fp8-pair words viewed as bf16 [128, NTILE, 4, 512])."""
    import ml_dtypes
    bf16 = ml_dtypes.bfloat16
    fp8 = ml_dtypes.float8_e4m3
    xb = xcore.astype(bf16)
    x8 = (xcore / 8.0).astype(fp8)  # [S, D]
    # word[p, t, s, n] = (x8[512t+n, 4p+s], x8[512t+n, 4p+s+512])
    arr = x8.reshape(NTILE, TOK, 2, 128, 4).transpose(3, 0, 4, 1, 2)
    arr = np.ascontiguousarray(arr)  # [128, NTILE, 4, TOK, 2] fp8
    xT = arr.view(np.uint16).reshape(128, NTILE, 4, TOK).view(bf16)
    return xb, xT


def kernel(**inputs):
    x = np.asarray(inputs["x"], dtype=np.float32)
    ln_g = np.asarray(inputs["ln_g"], dtype=np.float32)
    ln_b = np.asarray(inputs["ln_b"], dtype=np.float32)

    fast = (np.allclose(ln_g, 1.0, atol=1e-12) and
            np.allclose(ln_b, 0.0, atol=1e-12))
    if not fast:
        return _np_reference(
            x, *[np.asarray(inputs[k], dtype=np.float32) for k in
                 ("anchors", "ln_g", "ln_b", "W1", "b1", "W2", "b2", "cg",
                  "cb", "Wp", "bp", "gate")])

    params, use_const = _pack_params(
        inputs["anchors"], ln_g, inputs["W1"], inputs["b1"], inputs["W2"],
        inputs["b2"], inputs["cg"], inputs["cb"], inputs["Wp"], inputs["bp"],
        inputs["gate"])

    nc = _build_program(S, use_const)

    from concourse.bass_utils import run_bass_kernel_spmd
    in_maps = []
    for b in range(NCORES):
        m = dict(params)
        xb, xT = _pack_x(np.ascontiguousarray(x[b]))
        m["xb"] = xb
        m["xT"] = xT
        in_maps.append(m)
    res = run_bass_kernel_spmd(nc, in_maps, core_ids=list(range(NCORES)))
    out = np.stack([np.asarray(res.results[b]["out"]).astype(np.float32)
                    for b in range(NCORES)], axis=0)
    return out.reshape(B, S, D)


# revision 31
# speedup vs baseline: 1.2264x; 1.0258x over previous
"""Trainium2 Bass kernel for nn_ConstellationRelay (v3).

Computation (per token, D=1024, A=16 anchors, C=8 comps, dc=64):
  h   = l2norm(layernorm(x; ln_g, ln_b))
  tri = 1 - h @ l2norm(anchors).T                       (N, 16)
  u   = relu(einsum('nak,kae->nke', tri_g, W1) + b1)^2  (N, 8, 128)
  y   = layernorm_c(u @ W2 + b2; cg, cb)                (N, 8, 64)
  out = x + sigmoid(gate) * (y.flat @ Wp + bp)

Pure data-parallel over batch (one NeuronCore per batch row).  Fast path
requires ln_g==1, ln_b==0; general inputs fall back to numpy.

Structure:
  * x uploaded twice: token-major bf16 (stats + residual) + host-packed
    feature-major fp8 pairs (a0 DoubleRow rhs); output written bf16.
    LN mean-centering is folded into the anchors (a_c), so the on-device
    normalize pass and transpose of v1 are gone entirely.
  * 6-deep software pipeline, all PE inputs >= 1 slot old:
    load+stats+ee@t | a0@t+1 | expand/relu/square@t+2 | comp/yb/sqy/var/
    rstd@t+3 | sel/ycT@t+4 | proj/residual/out@t+5.
  * ee = 4/||x-mu|| from bn_stats + Sqrt + reciprocal; transposed via a
    tiny PE transpose and broadcast to [128,512] with 4 selector matmuls.
  * biasu applied inside the Act relu (relu(-up+biasu)); squared-relu and
    sqy are bf16 SBUF ops (DVE ~2x / Pool); yb (bf16) frees comp PSUM
    early; ycT = yb * rstd_bcast -> fp8 feeds the fp8-DR projection.
  * residual: 3 halves DVE scalar_tensor_tensor, 5 halves Act-copy +
    GpSimd add (GpSimd cannot read PSUM on TRN2).
  * rstd via Act-Sqrt + DVE reciprocal_approx_fast + Act bf16 copy.
  * PSUM: ps_y{eeb,a0p,yp,vst}=2, ps_exp{up}=3, ps_r{rbP,ee_t}=1,
    ps_u{ud}=2 banks.

Known environment facts (from traces): the NeuronCore is power-throttled
(avg util limit ~0.44), pinning the PE near its mid p-state (~1.2GHz) and
all per-op costs at ~2x the nominal model; runtime is a balanced mix of
PE (~82% busy), GpSimd, DVE and Act with ~23us of fixed startup+epilogue.
"""
import functools
import os
import sys

import numpy as np

for _p in ("/opt/trn_rl_repo",):
    if _p not in sys.path and os.path.isdir(_p):
        sys.path.insert(0, _p)

B, S, D = 8, 4096, 1024
A, C, DC = 16, 8, 64
APC = A // C
E2 = 2 * DC  # 128
NCORES = 8
TOK = 512
NTILE = S // TOK  # 8
NCH = TOK // 128  # 4
KD = D // 128  # 8
SP = 256.0  # fp8 scale on the folded projection matrix

RELU_DVE = (4,)          # ks whose squared-relu input runs on DVE (rest Act)
SQUARE_DVE = (0, 2, 5, 7)    # ks whose ubig square runs on DVE (rest Pool)
SQY_DVE = ()                 # js whose sqy square runs on DVE
SQY_ACT = ()                 # js whose sqy square runs on Act (rest Pool)
RES_PAIR = ((0, 1), (1, 0), (1, 1), (2, 1), (3, 1))  # Act-copy+Pool-add halves
USE_POOL_BCAST = False   # partition_broadcast needs partition-0 input


def _np_reference(x, anchors, ln_g, ln_b, W1, b1, W2, b2, cg, cb, Wp, bp, gate):
    x = x.astype(np.float32)
    N = x.shape[0] * x.shape[1]
    xf = x.reshape(N, D)
    mu = xf.mean(-1, keepdims=True)
    var = ((xf - mu) ** 2).mean(-1, keepdims=True)
    h = (xf - mu) / np.sqrt(var + 1e-5) * ln_g + ln_b
    h = h / np.maximum(np.linalg.norm(h, axis=-1, keepdims=True), 1e-12)
    a = anchors / np.maximum(np.linalg.norm(anchors, axis=-1, keepdims=True), 1e-12)
    tri = 1.0 - h @ a.T
    g = tri.reshape(N, APC, C)
    u = np.einsum("nak,kae->nke", g, W1) + b1
    u = np.square(np.maximum(u, 0.0))
    y = np.einsum("nke,ked->nkd", u, W2) + b2
    muy = y.mean(-1, keepdims=True)
    vy = ((y - muy) ** 2).mean(-1, keepdims=True)
    y = (y - muy) / np.sqrt(vy + 1e-5) * cg + cb
    upd = y.reshape(N, C * DC) @ Wp + bp
    sig = 1.0 / (1.0 + np.exp(-gate))
    return (xf + sig * upd).reshape(x.shape).astype(np.float32)


@functools.lru_cache(maxsize=4)
def _build_program(n_tokens=S, use_const=False):
    import concourse.bacc as bacc
    import concourse.mybir as mybir
    import concourse.tile as tile

    f32 = mybir.dt.float32
    bf16 = mybir.dt.bfloat16
    fp8 = mybir.dt.float8e4
    AF = mybir.ActivationFunctionType
    OP = mybir.AluOpType
    DR = mybir.MatmulPerfMode.DoubleRow

    ntile = n_tokens // TOK

    nc = bacc.Bacc("TRN2", target_bir_lowering=False, debug=False,
                   num_devices=NCORES)

    xb_d = nc.dram_tensor("xb", [n_tokens, D], bf16, kind="ExternalInput")
    xT_d = nc.dram_tensor("xT", [128, ntile, 4, TOK], bf16,
                          kind="ExternalInput")
    agt_d = nc.dram_tensor("agt", [128, 4, 2, 128], fp8, kind="ExternalInput")
    w1e_d = nc.dram_tensor("w1e", [128, KD, 128], bf16, kind="ExternalInput")
    biasu_d = nc.dram_tensor("biasu", [128, KD], f32, kind="ExternalInput")
    w2c_d = nc.dram_tensor("w2c", [128, C, DC], bf16, kind="ExternalInput")
    b2f_d = nc.dram_tensor("b2f", [128, 4], f32, kind="ExternalInput")
    vstl_d = nc.dram_tensor("vstl", [128, 4, C], bf16, kind="ExternalInput")
    sel_d = nc.dram_tensor("sel", [C, 4, 128], bf16, kind="ExternalInput")
    selc_d = nc.dram_tensor("selc", [4, 4, 128], bf16, kind="ExternalInput")
    ident_d = nc.dram_tensor("ident", [128, 128], bf16, kind="ExternalInput")
    wpf_d = nc.dram_tensor("wpf", [128, 2, 2, 2, 512], fp8,
                           kind="ExternalInput")
    cvec_d = nc.dram_tensor("cvec", [1, 2, 512], bf16, kind="ExternalInput") \
        if use_const else None
    out_d = nc.dram_tensor("out", [n_tokens, D], bf16, kind="ExternalOutput")
    ees_d = nc.dram_tensor("ees", [n_tokens // TOK, 4, 128], bf16,
                           kind="Internal")

    from contextlib import ExitStack

    with tile.TileContext(nc) as tc, ExitStack() as ctx:
        ctx.enter_context(nc.allow_low_precision(
            reason="update path is damped by sigmoid(gate)~0.047; fp8/bf16 "
                   "intermediates are well within the 2e-2 tolerance"))
        pp = ctx.enter_context(tc.tile_pool(name="params", bufs=1))
        px = ctx.enter_context(tc.tile_pool(name="px", bufs=2))
        psm = ctx.enter_context(tc.tile_pool(name="psm", bufs=2))
        # PSUM pools: 2 + 3 + 1 + 2 = 8 banks exactly.
        ps_y = ctx.enter_context(tc.tile_pool(name="ps_y", bufs=2,
                                              space="PSUM"))
        ps_exp = ctx.enter_context(tc.tile_pool(name="ps_exp", bufs=3,
                                                space="PSUM"))
        ps_r = ctx.enter_context(tc.tile_pool(name="ps_r", bufs=1,
                                              space="PSUM"))
        ps_u = ctx.enter_context(tc.tile_pool(name="ps_u", bufs=2,
                                              space="PSUM"))

        P = {}

        def load_params_a():
            P["ident"] = pp.tile([128, 128], bf16, name="ident")
            nc.sync.dma_start(out=P["ident"], in_=ident_d[:, :])
            P["agt"] = pp.tile([128, 4, 2, 128], fp8, name="agt")
            nc.sync.dma_start(out=P["agt"], in_=agt_d[:, :, :, :])
            P["selc"] = pp.tile([4, 4, 128], bf16, name="selc")
            nc.sync.dma_start(out=P["selc"], in_=selc_d[:, :, :])
            P["ctiny"] = pp.tile([128, 1], f32, name="ctiny")
            nc.vector.memset(P["ctiny"], 1e-20)
            P["cepsp"] = pp.tile([C, 1], f32, name="cepsp")
            nc.vector.memset(P["cepsp"], 1e-5)

        def load_params_b():
            P["w1e"] = pp.tile([128, KD, 128], bf16, name="w1e")
            nc.sync.dma_start(out=P["w1e"], in_=w1e_d[:, :, :])
            P["biasu"] = pp.tile([128, KD], f32, name="biasu")
            nc.sync.dma_start(out=P["biasu"], in_=biasu_d[:, :])
            P["w2c"] = pp.tile([128, C, DC], bf16, name="w2c")
            nc.sync.dma_start(out=P["w2c"], in_=w2c_d[:, :, :])
            P["b2f"] = pp.tile([128, 4], f32, name="b2f")
            nc.sync.dma_start(out=P["b2f"], in_=b2f_d[:, :])
            P["vstl"] = pp.tile([128, 4, C], bf16, name="vstl")
            nc.sync.dma_start(out=P["vstl"], in_=vstl_d[:, :, :])

        def load_params_c():
            P["sel"] = pp.tile([C, 4, 128], bf16, name="sel")
            nc.sync.dma_start(out=P["sel"], in_=sel_d[:, :, :])
            P["wpf"] = pp.tile([128, 2, 2, 2, 512], fp8, name="wpf")
            nc.sync.dma_start(out=P["wpf"], in_=wpf_d[:, :, :, :, :])
            if use_const:
                P["cvec"] = pp.tile([1, 2, 512], bf16, name="cvec")
                nc.sync.dma_start(out=P["cvec"], in_=cvec_d[:, :, :])
                P["ones1"] = pp.tile([1, 128], bf16, name="ones1")
                nc.vector.memset(P["ones1"], 1.0)

        def stage_load(t):
            row0 = t * TOK
            xt = px.tile([128, NCH, D], bf16, tag="xt", bufs=6, name=f"xt{t}")
            nc.sync.dma_start(
                out=xt,
                in_=xb_d[row0: row0 + TOK, :].rearrange(
                    "(c p) d -> p c d", p=128))
            xT = px.tile([128, 4, TOK], bf16, tag="xT", bufs=3, name=f"xT{t}")
            nc.sync.dma_start(out=xT, in_=xT_d[:, t, :, :])
            return xt, xT

        def stage_stats_bn(t, xt):
            mv = psm.tile([128, NCH, 2], f32, tag="mv", name=f"mv{t}")
            for cch in range(NCH):
                st = psm.tile([128, 2, 6], f32, tag="st")
                xr = xt[:, cch, :].rearrange("p (s f) -> p s f", s=2)
                nc.vector.bn_stats(out=st[:, 0, :], in_=xr[:, 0, :])
                nc.vector.bn_stats(out=st[:, 1, :], in_=xr[:, 1, :])
                nc.vector.bn_aggr(out=mv[:, cch, :], in_=st)
            # ee = 4/||x-mu|| = 1/sqrt(D/16 * var)
            sd = psm.tile([128, NCH], f32, tag="sd")
            nc.scalar.activation(sd, mv[:, :, 1], AF.Sqrt, bias=P["ctiny"],
                                 scale=float(D) / 16.0)
            eebf = psm.tile([128, NCH], bf16, tag="eebf", name=f"eebf{t}")
            nc.vector.reciprocal(eebf, sd)
            return eebf

        def stage_stats_tp(t, eebf):
            ee_t = ps_r.tile([NCH, 128], bf16, tag="r", name=f"eet{t}")
            nc.tensor.transpose(ee_t, eebf, P["ident"])
            eeT = psm.tile([NCH, 128], bf16, tag="eeT", bufs=2,
                           name=f"eeT{t}")
            nc.scalar.copy(out=eeT, in_=ee_t)
            nc.sync.dma_start(out=ees_d[t, :, :], in_=eeT)
            eesb = px.tile([128, TOK], bf16, tag="eesb", bufs=3,
                           name=f"eesb{t}")
            for cch in range(NCH):
                nc.sync.dma_start(
                    out=eesb[:, cch * 128:(cch + 1) * 128],
                    in_=ees_d[t, cch, :].unsqueeze(0).to_broadcast(
                        (128, 128)))
            return eesb

        def stage_a0(t, xT, eesb):
            a0p = ps_y.tile([128, TOK], f32, tag="y", name=f"a0p{t}")
            for s in range(4):
                rhs8 = xT[:, s, :].bitcast(fp8).rearrange(
                    "p (n i) -> p i n", i=2)
                nc.tensor.matmul(a0p, lhsT=P["agt"][:, s, :, :], rhs=rhs8,
                                 start=(s == 0), stop=(s == 3), perf_mode=DR)
            a0s = px.tile([128, TOK], bf16, tag="a0s", bufs=2, name=f"a0s{t}")
            nc.vector.tensor_mul(a0s, a0p, eesb)
            return a0s

        def stage_expand(t, a0s):
            rb = px.tile([128, KD, TOK], bf16, tag="rb", bufs=2)
            ubig = px.tile([128, KD, TOK], bf16, tag="ubig", bufs=3,
                           name=f"ubig{t}")
            for kg in range(2):
                ups = []
                for r in range(4):
                    k = 4 * kg + r
                    up = ps_exp.tile([128, TOK], f32, tag="exp")
                    nc.tensor.matmul(
                        up, lhsT=P["w1e"][32 * r:32 * r + A, k, :],
                        rhs=a0s[32 * r:32 * r + A, :],
                        start=True, stop=True,
                        tile_position=(32 * r, 0))
                    ups.append(up)
                for r in range(4):
                    k = 4 * kg + r
                    if k in RELU_DVE:
                        # min(up-biasu, 0) = -relu(biasu-up); square kills sign
                        nc.vector.tensor_scalar(
                            out=rb[:, k, :], in0=ups[r],
                            scalar1=P["biasu"][:, k:k + 1], scalar2=0.0,
                            op0=OP.subtract, op1=OP.min)
                    else:
                        nc.scalar.activation(rb[:, k, :], ups[r], AF.Relu,
                                             bias=P["biasu"][:, k:k + 1],
                                             scale=-1.0)
                for r in range(4):
                    k = 4 * kg + r
                    eng = nc.vector if k in SQUARE_DVE else nc.gpsimd
                    eng.tensor_mul(ubig[:, k, :], rb[:, k, :], rb[:, k, :])
            return ubig

        def stage_comp(t, ubig):
            yb = px.tile([128, 4, TOK], bf16, tag="yb", bufs=3, name=f"yb{t}")
            for j in range(4):
                yp = ps_y.tile([128, TOK], f32, tag="y")
                nc.tensor.matmul(yp[0:64, :], lhsT=P["w2c"][:, 2 * j, :],
                                 rhs=ubig[:, 2 * j, :], start=True, stop=True)
                nc.tensor.matmul(yp[64:128, :], lhsT=P["w2c"][:, 2 * j + 1, :],
                                 rhs=ubig[:, 2 * j + 1, :], start=True,
                                 stop=True, tile_position=(0, 64))
                nc.scalar.activation(yb[:, j, :], yp, AF.Identity,
                                     bias=P["b2f"][:, j:j + 1], scale=1.0)
            sqy = px.tile([128, 4, TOK], bf16, tag="sqy", bufs=2,
                          name=f"sqy{t}")
            for j in range(4):
                nc.vector.tensor_mul(sqy[:, j, :], yb[:, j, :], yb[:, j, :])
            return yb, sqy

        def stage_var(t, sqy):
            vst = ps_y.tile([C, TOK], f32, tag="y", name=f"vst{t}")
            for j in range(4):
                nc.tensor.matmul(vst, lhsT=P["vstl"][:, j, :],
                                 rhs=sqy[:, j, :],
                                 start=(j == 0), stop=(j == 3))
            sd2 = psm.tile([C, TOK], f32, tag="sd2", bufs=2)
            nc.scalar.activation(sd2, vst, AF.Sqrt, bias=P["cepsp"],
                                 scale=1.0)
            rr = psm.tile([C, TOK], f32, tag="rr", bufs=2)
            nc.vector.reciprocal_approx_fast(out=rr, in_=sd2)
            rrb = psm.tile([C, TOK], bf16, tag="rrb", bufs=2, name=f"rrb{t}")
            nc.scalar.copy(out=rrb, in_=rr)
            return rrb

        def stage_back_sel(t, yb, rrb):
            ycT = px.tile([128, 4, TOK], fp8, tag="ycT", bufs=3,
                          name=f"ycT{t}")
            for j in range(4):
                # alternate PSUM pools: ps_u's banks are idle early-slot
                # (proj runs late), letting two sel/ycT chains run in
                # parallel instead of serializing through ps_r's one bank
                pool = ps_u if j % 2 else ps_r
                rbP = pool.tile([128, TOK], f32,
                                tag=("ud" if j % 2 else "r"),
                                name=f"rbP{t}_{j}")
                nc.tensor.matmul(rbP, lhsT=P["sel"][:, j, :], rhs=rrb,
                                 start=True, stop=True)
                nc.vector.tensor_mul(ycT[:, j, :], yb[:, j, :], rbP)
            return ycT

        def stage_back_proj(t, xt, ycT):
            row0 = t * TOK
            for cch in range(NCH):
                osb = px.tile([128, D], bf16, tag="osb", bufs=3,
                              name=f"osb{t}_{cch}")
                for hf in range(2):
                    ud = ps_u.tile([128, 512], f32, tag="ud",
                                   name=f"ud{t}_{cch}_{hf}")
                    for m in range(2):
                        nc.tensor.matmul(
                            ud,
                            lhsT=ycT[:, 2 * m:2 * m + 2,
                                     cch * 128:(cch + 1) * 128],
                            rhs=P["wpf"][:, m, :, hf, :],
                            start=(m == 0),
                            stop=(m == 1 and not use_const),
                            perf_mode=DR)
                    if use_const:
                        nc.tensor.matmul(ud, lhsT=P["ones1"],
                                         rhs=P["cvec"][:, hf, :],
                                         start=False, stop=True)
                    xs = xt[:, cch, hf * 512:(hf + 1) * 512]
                    dst = osb[:, hf * 512:(hf + 1) * 512]
                    if (cch, hf) in RES_PAIR:
                        # Pool can't read PSUM: Act scales ud into SBUF,
                        # Pool adds the residual
                        uph = px.tile([128, 512], bf16, tag="uph", bufs=3,
                                      name=f"uph{t}_{cch}_{hf}")
                        nc.scalar.activation(uph, ud, AF.Identity,
                                             scale=1.0 / SP)
                        nc.gpsimd.tensor_tensor(out=dst, in0=uph, in1=xs,
                                                op=OP.add)
                    else:
                        nc.vector.scalar_tensor_tensor(
                            out=dst, in0=ud, scalar=1.0 / SP,
                            in1=xs, op0=OP.mult, op1=OP.add)
                nc.sync.dma_start(
                    out=out_d[row0 + cch * 128: row0 + (cch + 1) * 128, :],
                    in_=osb)

        ld = {}
        ee = {}
        a0 = {}
        ub = {}
        cmp_ = {}
        yc = {}
        # pipeline: load+stats@t a0@t+1 expand@t+2 comp/var@t+3 sel/ycT@t+4
        # proj/residual/out@t+5
        for s in range(ntile + 5):
            if s < ntile:
                ld[s] = stage_load(s)
            if s == 0:
                load_params_a()
            if s == 1:
                load_params_b()
            if s == 2:
                load_params_c()
            if 1 <= s <= ntile:
                t = s - 1
                a0[t] = stage_a0(t, ld[t][1], ee.pop(t))
            if 4 <= s <= ntile + 3:
                t = s - 4
                ybb, rrbb = cmp_.pop(t)
                yc[t] = stage_back_sel(t, ybb, rrbb)
            if 3 <= s <= ntile + 2:
                t = s - 3
                ybb2, sqy2 = stage_comp(t, ub.pop(t))
            if 2 <= s <= ntile + 1:
                t = s - 2
                ub[t] = stage_expand(t, a0.pop(t))
            if 5 <= s <= ntile + 4:
                t = s - 5
                stage_back_proj(t, ld.pop(t)[0], yc.pop(t))
            if 3 <= s <= ntile + 2:
                t = s - 3
                cmp_[t] = (ybb2, stage_var(t, sqy2))
            if s < ntile:
                t = s
                eebf_ = stage_stats_bn(t, ld[t][0])
                ee[t] = stage_stats_tp(t, eebf_)

    nc.compile()
    return nc


def _pack_params(anchors, ln_g, W1, b1, W2, b2, cg, cb, Wp, bp, gate):
    import ml_dtypes
    f32 = np.float32
    bf16 = ml_dtypes.bfloat16
    fp8 = ml_dtypes.float8_e4m3

    anchors = anchors.astype(f32)
    an = anchors / np.maximum(
        np.linalg.norm(anchors.astype(np.float64), axis=1, keepdims=True),
        1e-12).astype(f32)
    ag = an * ln_g[None, :].astype(f32)  # [A, D]
    a_c = ag - ag.mean(axis=1, keepdims=True)  # fold LN mean-centering

    # agt[p, s, i, 32r+m] = 2*a_c[m, 4p+s+512i], 4 row replicas
    agt = np.zeros((128, 4, 2, 128), f32)
    ww = np.arange(512)
    pidx, sidx = ww // 4, ww % 4
    for i in range(2):
        for r in range(4):
            agt[pidx, sidx, i, 32 * r:32 * r + A] = 2.0 * a_c.T[ww + 512 * i]

    # W1exp[m, k, e] = W1[k, j, e] iff m == j*C + k
    W1 = W1.astype(f32)
    w1exp = np.zeros((A, C, E2), f32)
    for m in range(A):
        j, k2 = m // C, m % C
        w1exp[m, k2, :] = W1[k2, j, :]
    biasu_np = w1exp.sum(axis=0) + b1.astype(f32)  # [C, E2]
    w1e = np.zeros((128, KD, 128), f32)
    for k in range(KD):
        r = k % 4
        w1e[32 * r:32 * r + A, k, :] = w1exp[:, k, :]
    biasu_sb = np.ascontiguousarray(biasu_np.T)  # [128, KD]

    W2 = W2.astype(f32)
    w2cent = W2 - W2.mean(axis=2, keepdims=True)
    w2c = np.transpose(w2cent, (1, 0, 2)).copy()  # [128, C, 64]
    b2c = b2.astype(f32) - b2.astype(f32).mean(axis=1, keepdims=True)

    b2f = np.zeros((128, 4), f32)
    for j in range(4):
        for p in range(128):
            kk = 2 * j + p // 64
            b2f[p, j] = b2c[kk, p % 64]

    vstl = np.zeros((128, 4, C), f32)
    for j in range(4):
        for p in range(128):
            vstl[p, j, 2 * j + p // 64] = 1.0 / DC

    sig = (1.0 / (1.0 + np.exp(-gate.astype(np.float64)))).astype(f32)
    wpfold = (cg.astype(f32).reshape(C * DC, 1) * Wp.astype(f32)) * sig[None, :]
    wpf = np.ascontiguousarray(
        (SP * wpfold).reshape(2, 2, 128, 2, 512).transpose(2, 0, 1, 3, 4))

    const = (cb.astype(f32).reshape(-1) @ Wp.astype(f32) + bp.astype(f32)) * sig
    use_const = bool(np.max(np.abs(const)) > 0)

    sel_np = np.zeros((C, 4, 128), f32)
    for j in range(4):
        sel_np[2 * j, j, 0:64] = 1.0
        sel_np[2 * j + 1, j, 64:128] = 1.0

    selc = np.zeros((4, 4, 128), f32)
    for c in range(4):
        selc[c, c, :] = 1.0

    params = dict(
        agt=agt.astype(fp8),
        w1e=w1e.astype(bf16),
        biasu=biasu_sb.astype(f32),
        w2c=w2c.astype(bf16),
        b2f=b2f.astype(f32),
        vstl=vstl.astype(bf16),
        sel=sel_np.astype(bf16),
        selc=selc.astype(bf16),
        ident=np.eye(128, dtype=f32).astype(bf16),
        idsp=(SP * np.eye(128, dtype=f32)).astype(bf16),
        wpf=wpf.astype(fp8),
    )
    if use_const:
        params["cvec"] = (SP * const).reshape(1, 2, 512).astype(bf16)
    return params, use_const


def _pack_x(xcore):
    """Pack one core's x [S, D] f32 into (token-major bf16, feature-major
    fp8-pair words viewed as bf16 [128, NTILE, 4, 512])."""
    import ml_dtypes
    bf16 = ml_dtypes.bfloat16
    fp8 = ml_dtypes.float8_e4m3
    xb = xcore.astype(bf16)
    x8 = (xcore / 8.0).astype(fp8)  # [S, D]
    # word[p, t, s, n] = (x8[512t+n, 4p+s], x8[512t+n, 4p+s+512])
    arr = x8.reshape(NTILE, TOK, 2, 128, 4).transpose(3, 0, 4, 1, 2)
    arr = np.ascontiguousarray(arr)  # [128, NTILE, 4, TOK, 2] fp8
    xT = arr.view(np.uint16).reshape(128, NTILE, 4, TOK).view(bf16)
    return xb, xT


def kernel(**inputs):
    x = np.asarray(inputs["x"], dtype=np.float32)
    ln_g = np.asarray(inputs["ln_g"], dtype=np.float32)
    ln_b = np.asarray(inputs["ln_b"], dtype=np.float32)

    fast = (np.allclose(ln_g, 1.0, atol=1e-12) and
            np.allclose(ln_b, 0.0, atol=1e-12))
    if not fast:
        return _np_reference(
            x, *[np.asarray(inputs[k], dtype=np.float32) for k in
                 ("anchors", "ln_g", "ln_b", "W1", "b1", "W2", "b2", "cg",
                  "cb", "Wp", "bp", "gate")])

    params, use_const = _pack_params(
        inputs["anchors"], ln_g, inputs["W1"], inputs["b1"], inputs["W2"],
        inputs["b2"], inputs["cg"], inputs["cb"], inputs["Wp"], inputs["bp"],
        inputs["gate"])

    nc = _build_program(S, use_const)

    from concourse.bass_utils import run_bass_kernel_spmd
    in_maps = []
    for b in range(NCORES):
        m = dict(params)
        xb, xT = _pack_x(np.ascontiguousarray(x[b]))
        m["xb"] = xb
        m["xT"] = xT
        in_maps.append(m)
    res = run_bass_kernel_spmd(nc, in_maps, core_ids=list(range(NCORES)))
    out = np.stack([np.asarray(res.results[b]["out"]).astype(np.float32)
                    for b in range(NCORES)], axis=0)
    return out.reshape(B, S, D)


# revision 32
# speedup vs baseline: 1.4284x; 1.1647x over previous
"""Trainium2 Bass kernel for nn_ConstellationRelay (v3).

Computation (per token, D=1024, A=16 anchors, C=8 comps, dc=64):
  h   = l2norm(layernorm(x; ln_g, ln_b))
  tri = 1 - h @ l2norm(anchors).T                       (N, 16)
  u   = relu(einsum('nak,kae->nke', tri_g, W1) + b1)^2  (N, 8, 128)
  y   = layernorm_c(u @ W2 + b2; cg, cb)                (N, 8, 64)
  out = x + sigmoid(gate) * (y.flat @ Wp + bp)

Pure data-parallel over batch (one NeuronCore per batch row).  Fast path
requires ln_g==1, ln_b==0; general inputs fall back to numpy.

Structure:
  * x uploaded twice: token-major bf16 (stats + residual) + host-packed
    feature-major fp8 pairs (a0 DoubleRow rhs); output written bf16.
    LN mean-centering is folded into the anchors (a_c), so the on-device
    normalize pass and transpose of v1 are gone entirely.
  * 6-deep software pipeline, all PE inputs >= 1 slot old:
    load+stats+ee@t | a0@t+1 | expand/relu/square@t+2 | comp/yb/sqy/var/
    rstd@t+3 | sel/ycT@t+4 | proj/residual/out@t+5.
  * ee = 4/||x-mu|| from bn_stats + Sqrt + reciprocal; transposed via a
    tiny PE transpose and broadcast to [128,512] with 4 selector matmuls.
  * biasu applied inside the Act relu (relu(-up+biasu)); squared-relu and
    sqy are bf16 SBUF ops (DVE ~2x / Pool); yb (bf16) frees comp PSUM
    early; ycT = yb * rstd_bcast -> fp8 feeds the fp8-DR projection.
  * residual: 3 halves DVE scalar_tensor_tensor, 5 halves Act-copy +
    GpSimd add (GpSimd cannot read PSUM on TRN2).
  * rstd via Act-Sqrt + DVE reciprocal_approx_fast + Act bf16 copy.
  * PSUM: ps_y{eeb,a0p,yp,vst}=2, ps_exp{up}=3, ps_r{rbP,ee_t}=1,
    ps_u{ud}=2 banks.

Known environment facts (from traces): the NeuronCore is power-throttled
(avg util limit ~0.44), pinning the PE near its mid p-state (~1.2GHz) and
all per-op costs at ~2x the nominal model; runtime is a balanced mix of
PE (~82% busy), GpSimd, DVE and Act with ~23us of fixed startup+epilogue.
"""
import functools
import os
import sys

import numpy as np

for _p in ("/opt/trn_rl_repo",):
    if _p not in sys.path and os.path.isdir(_p):
        sys.path.insert(0, _p)

B, S, D = 8, 4096, 1024
A, C, DC = 16, 8, 64
APC = A // C
E2 = 2 * DC  # 128
NCORES = 8
TOK = 512
NTILE = S // TOK  # 8
NCH = TOK // 128  # 4
KD = D // 128  # 8
SP = 256.0  # fp8 scale on the folded projection matrix

RELU_DVE = (4,)          # ks whose squared-relu input runs on DVE (rest Act)
SQUARE_DVE = (0, 2, 5, 7)    # ks whose ubig square runs on DVE (rest Pool)
SQY_DVE = ()                 # js whose sqy square runs on DVE
SQY_ACT = ()                 # js whose sqy square runs on Act (rest Pool)
RES_PAIR = ((0, 1), (1, 0), (1, 1), (2, 1), (3, 1))  # Act-copy+Pool-add halves
USE_POOL_BCAST = False   # partition_broadcast needs partition-0 input


def _np_reference(x, anchors, ln_g, ln_b, W1, b1, W2, b2, cg, cb, Wp, bp, gate):
    x = x.astype(np.float32)
    N = x.shape[0] * x.shape[1]
    xf = x.reshape(N, D)
    mu = xf.mean(-1, keepdims=True)
    var = ((xf - mu) ** 2).mean(-1, keepdims=True)
    h = (xf - mu) / np.sqrt(var + 1e-5) * ln_g + ln_b
    h = h / np.maximum(np.linalg.norm(h, axis=-1, keepdims=True), 1e-12)
    a = anchors / np.maximum(np.linalg.norm(anchors, axis=-1, keepdims=True), 1e-12)
    tri = 1.0 - h @ a.T
    g = tri.reshape(N, APC, C)
    u = np.einsum("nak,kae->nke", g, W1) + b1
    u = np.square(np.maximum(u, 0.0))
    y = np.einsum("nke,ked->nkd", u, W2) + b2
    muy = y.mean(-1, keepdims=True)
    vy = ((y - muy) ** 2).mean(-1, keepdims=True)
    y = (y - muy) / np.sqrt(vy + 1e-5) * cg + cb
    upd = y.reshape(N, C * DC) @ Wp + bp
    sig = 1.0 / (1.0 + np.exp(-gate))
    return (xf + sig * upd).reshape(x.shape).astype(np.float32)


@functools.lru_cache(maxsize=4)
def _build_program(n_tokens=S, use_const=False):
    import concourse.bacc as bacc
    import concourse.mybir as mybir
    import concourse.tile as tile

    f32 = mybir.dt.float32
    bf16 = mybir.dt.bfloat16
    fp8 = mybir.dt.float8e4
    AF = mybir.ActivationFunctionType
    OP = mybir.AluOpType
    DR = mybir.MatmulPerfMode.DoubleRow

    ntile = n_tokens // TOK

    nc = bacc.Bacc("TRN2", target_bir_lowering=False, debug=False,
                   num_devices=NCORES)

    xb_d = nc.dram_tensor("xb", [n_tokens, D], bf16, kind="ExternalInput")
    xT_d = nc.dram_tensor("xT", [128, ntile, 4, TOK], bf16,
                          kind="ExternalInput")
    agt_d = nc.dram_tensor("agt", [128, 4, 2, 128], fp8, kind="ExternalInput")
    w1e_d = nc.dram_tensor("w1e", [128, KD, 128], bf16, kind="ExternalInput")
    biasu_d = nc.dram_tensor("biasu", [128, KD], f32, kind="ExternalInput")
    w2c_d = nc.dram_tensor("w2c", [128, C, DC], bf16, kind="ExternalInput")
    b2f_d = nc.dram_tensor("b2f", [128, 4], f32, kind="ExternalInput")
    vstl_d = nc.dram_tensor("vstl", [128, 4, C], bf16, kind="ExternalInput")
    sel_d = nc.dram_tensor("sel", [C, 4, 128], bf16, kind="ExternalInput")
    selc_d = nc.dram_tensor("selc", [4, 4, 128], bf16, kind="ExternalInput")
    ident_d = nc.dram_tensor("ident", [128, 128], bf16, kind="ExternalInput")
    wpf_d = nc.dram_tensor("wpf", [128, 2, 2, 2, 512], fp8,
                           kind="ExternalInput")
    cvec_d = nc.dram_tensor("cvec", [1, 2, 512], bf16, kind="ExternalInput") \
        if use_const else None
    out_d = nc.dram_tensor("out", [n_tokens, D], bf16, kind="ExternalOutput")
    ees_d = nc.dram_tensor("ees", [n_tokens // TOK, 4, 128], bf16,
                           kind="Internal")

    from contextlib import ExitStack

    with tile.TileContext(nc) as tc, ExitStack() as ctx:
        ctx.enter_context(nc.allow_low_precision(
            reason="update path is damped by sigmoid(gate)~0.047; fp8/bf16 "
                   "intermediates are well within the 2e-2 tolerance"))
        pp = ctx.enter_context(tc.tile_pool(name="params", bufs=1))
        px = ctx.enter_context(tc.tile_pool(name="px", bufs=2))
        psm = ctx.enter_context(tc.tile_pool(name="psm", bufs=2))
        # PSUM pools: 2 + 3 + 1 + 2 = 8 banks exactly.
        ps_y = ctx.enter_context(tc.tile_pool(name="ps_y", bufs=2,
                                              space="PSUM"))
        ps_exp = ctx.enter_context(tc.tile_pool(name="ps_exp", bufs=3,
                                                space="PSUM"))
        ps_r = ctx.enter_context(tc.tile_pool(name="ps_r", bufs=1,
                                              space="PSUM"))
        ps_u = ctx.enter_context(tc.tile_pool(name="ps_u", bufs=2,
                                              space="PSUM"))

        P = {}

        def load_params_a():
            P["ident"] = pp.tile([128, 128], bf16, name="ident")
            nc.sync.dma_start(out=P["ident"], in_=ident_d[:, :])
            P["agt"] = pp.tile([128, 4, 2, 128], fp8, name="agt")
            nc.sync.dma_start(out=P["agt"], in_=agt_d[:, :, :, :])
            P["selc"] = pp.tile([4, 4, 128], bf16, name="selc")
            nc.sync.dma_start(out=P["selc"], in_=selc_d[:, :, :])
            P["ctiny"] = pp.tile([128, 1], f32, name="ctiny")
            nc.vector.memset(P["ctiny"], 1e-20)
            P["cepsp"] = pp.tile([C, 1], f32, name="cepsp")
            nc.vector.memset(P["cepsp"], 1e-5)

        def load_params_b():
            P["w1e"] = pp.tile([128, KD, 128], bf16, name="w1e")
            nc.sync.dma_start(out=P["w1e"], in_=w1e_d[:, :, :])
            P["biasu"] = pp.tile([128, KD], f32, name="biasu")
            nc.sync.dma_start(out=P["biasu"], in_=biasu_d[:, :])
            P["w2c"] = pp.tile([128, C, DC], bf16, name="w2c")
            nc.sync.dma_start(out=P["w2c"], in_=w2c_d[:, :, :])
            P["b2f"] = pp.tile([128, 4], f32, name="b2f")
            nc.sync.dma_start(out=P["b2f"], in_=b2f_d[:, :])
            P["vstl"] = pp.tile([128, 4, C], bf16, name="vstl")
            nc.sync.dma_start(out=P["vstl"], in_=vstl_d[:, :, :])

        def load_params_c():
            P["sel"] = pp.tile([C, 4, 128], bf16, name="sel")
            nc.sync.dma_start(out=P["sel"], in_=sel_d[:, :, :])
            P["wpf"] = pp.tile([128, 2, 2, 2, 512], fp8, name="wpf")
            nc.sync.dma_start(out=P["wpf"], in_=wpf_d[:, :, :, :, :])
            if use_const:
                P["cvec"] = pp.tile([1, 2, 512], bf16, name="cvec")
                nc.sync.dma_start(out=P["cvec"], in_=cvec_d[:, :, :])
                P["ones1"] = pp.tile([1, 128], bf16, name="ones1")
                nc.vector.memset(P["ones1"], 1.0)

        def stage_load(t):
            row0 = t * TOK
            xt = px.tile([128, NCH, D], bf16, tag="xt", bufs=6, name=f"xt{t}")
            nc.sync.dma_start(
                out=xt,
                in_=xb_d[row0: row0 + TOK, :].rearrange(
                    "(c p) d -> p c d", p=128))
            xT = px.tile([128, 4, TOK], bf16, tag="xT", bufs=3, name=f"xT{t}")
            nc.sync.dma_start(out=xT, in_=xT_d[:, t, :, :])
            return xt, xT

        def stage_stats_bn(t, xt):
            mv = psm.tile([128, NCH, 2], f32, tag="mv", name=f"mv{t}")
            for cch in range(NCH):
                st = psm.tile([128, 2, 6], f32, tag="st")
                xr = xt[:, cch, :].rearrange("p (s f) -> p s f", s=2)
                nc.vector.bn_stats(out=st[:, 0, :], in_=xr[:, 0, :])
                nc.vector.bn_stats(out=st[:, 1, :], in_=xr[:, 1, :])
                nc.vector.bn_aggr(out=mv[:, cch, :], in_=st)
            # ee = 4/||x-mu|| = 1/sqrt(D/16 * var)
            sd = psm.tile([128, NCH], f32, tag="sd")
            nc.scalar.activation(sd, mv[:, :, 1], AF.Sqrt, bias=P["ctiny"],
                                 scale=float(D) / 16.0)
            eebf = psm.tile([128, NCH], bf16, tag="eebf", name=f"eebf{t}")
            nc.vector.reciprocal(eebf, sd)
            return eebf

        def stage_stats_tp(t, eebf):
            ee_t = ps_r.tile([NCH, 128], bf16, tag="r", name=f"eet{t}")
            nc.tensor.transpose(ee_t, eebf, P["ident"])
            eeT = psm.tile([NCH, 128], bf16, tag="eeT", bufs=2,
                           name=f"eeT{t}")
            nc.scalar.copy(out=eeT, in_=ee_t)
            nc.sync.dma_start(out=ees_d[t, :, :], in_=eeT)
            eesb = px.tile([128, TOK], bf16, tag="eesb", bufs=3,
                           name=f"eesb{t}")
            for cch in range(NCH):
                nc.sync.dma_start(
                    out=eesb[:, cch * 128:(cch + 1) * 128],
                    in_=ees_d[t, cch, :].unsqueeze(0).to_broadcast(
                        (128, 128)))
            return eesb

        def stage_a0(t, xT, eesb):
            a0p = ps_y.tile([128, TOK], f32, tag="y", name=f"a0p{t}")
            for s in range(4):
                rhs8 = xT[:, s, :].bitcast(fp8).rearrange(
                    "p (n i) -> p i n", i=2)
                nc.tensor.matmul(a0p, lhsT=P["agt"][:, s, :, :], rhs=rhs8,
                                 start=(s == 0), stop=(s == 3), perf_mode=DR)
            a0s = px.tile([128, TOK], bf16, tag="a0s", bufs=2, name=f"a0s{t}")
            nc.vector.tensor_mul(a0s, a0p, eesb)
            return a0s

        def stage_expand(t, a0s):
            rb = px.tile([128, KD, TOK], bf16, tag="rb", bufs=2)
            ubig = px.tile([128, KD, TOK], bf16, tag="ubig", bufs=3,
                           name=f"ubig{t}")
            for kg in range(2):
                ups = []
                for r in range(4):
                    k = 4 * kg + r
                    up = ps_exp.tile([128, TOK], f32, tag="exp")
                    nc.tensor.matmul(
                        up, lhsT=P["w1e"][32 * r:32 * r + A, k, :],
                        rhs=a0s[32 * r:32 * r + A, :],
                        start=True, stop=True,
                        tile_position=(32 * r, 0))
                    ups.append(up)
                for r in range(4):
                    k = 4 * kg + r
                    if k in RELU_DVE:
                        # min(up-biasu, 0) = -relu(biasu-up); square kills sign
                        nc.vector.tensor_scalar(
                            out=rb[:, k, :], in0=ups[r],
                            scalar1=P["biasu"][:, k:k + 1], scalar2=0.0,
                            op0=OP.subtract, op1=OP.min)
                    else:
                        nc.scalar.activation(rb[:, k, :], ups[r], AF.Relu,
                                             bias=P["biasu"][:, k:k + 1],
                                             scale=-1.0)
                for r in range(4):
                    k = 4 * kg + r
                    eng = nc.vector if k in SQUARE_DVE else nc.gpsimd
                    eng.tensor_mul(ubig[:, k, :], rb[:, k, :], rb[:, k, :])
            return ubig

        def stage_comp(t, ubig):
            yb = px.tile([128, 4, TOK], bf16, tag="yb", bufs=3, name=f"yb{t}")
            for j in range(4):
                yp = ps_y.tile([128, TOK], f32, tag="y")
                nc.tensor.matmul(yp[0:64, :], lhsT=P["w2c"][:, 2 * j, :],
                                 rhs=ubig[:, 2 * j, :], start=True, stop=True)
                nc.tensor.matmul(yp[64:128, :], lhsT=P["w2c"][:, 2 * j + 1, :],
                                 rhs=ubig[:, 2 * j + 1, :], start=True,
                                 stop=True, tile_position=(0, 64))
                nc.scalar.activation(yb[:, j, :], yp, AF.Identity,
                                     bias=P["b2f"][:, j:j + 1], scale=1.0)
            sqy = px.tile([128, 4, TOK], bf16, tag="sqy", bufs=2,
                          name=f"sqy{t}")
            for j in range(4):
                nc.vector.tensor_mul(sqy[:, j, :], yb[:, j, :], yb[:, j, :])
            return yb, sqy

        def stage_var(t, sqy):
            vst = ps_y.tile([C, TOK], f32, tag="y", name=f"vst{t}")
            for j in range(4):
                nc.tensor.matmul(vst, lhsT=P["vstl"][:, j, :],
                                 rhs=sqy[:, j, :],
                                 start=(j == 0), stop=(j == 3))
            sd2 = psm.tile([C, TOK], f32, tag="sd2", bufs=2)
            nc.scalar.activation(sd2, vst, AF.Sqrt, bias=P["cepsp"],
                                 scale=1.0)
            rr = psm.tile([C, TOK], f32, tag="rr", bufs=2)
            nc.vector.reciprocal_approx_fast(out=rr, in_=sd2)
            rrb = psm.tile([C, TOK], bf16, tag="rrb", bufs=2, name=f"rrb{t}")
            nc.scalar.copy(out=rrb, in_=rr)
            return rrb

        def stage_back_sel(t, yb, rrb):
            ycT = px.tile([128, 4, TOK], fp8, tag="ycT", bufs=3,
                          name=f"ycT{t}")
            for j in range(4):
                rbP = ps_r.tile([128, TOK], f32, tag="r")
                nc.tensor.matmul(rbP, lhsT=P["sel"][:, j, :], rhs=rrb,
                                 start=True, stop=True)
                nc.vector.tensor_mul(ycT[:, j, :], yb[:, j, :], rbP)
            return ycT

        def stage_back_proj(t, xt, ycT):
            row0 = t * TOK
            for cch in range(NCH):
                osb = px.tile([128, D], bf16, tag="osb", bufs=3,
                              name=f"osb{t}_{cch}")
                for hf in range(2):
                    ud = ps_u.tile([128, 512], f32, tag="ud",
                                   name=f"ud{t}_{cch}_{hf}")
                    for m in range(2):
                        nc.tensor.matmul(
                            ud,
                            lhsT=ycT[:, 2 * m:2 * m + 2,
                                     cch * 128:(cch + 1) * 128],
                            rhs=P["wpf"][:, m, :, hf, :],
                            start=(m == 0),
                            stop=(m == 1 and not use_const),
                            perf_mode=DR)
                    if use_const:
                        nc.tensor.matmul(ud, lhsT=P["ones1"],
                                         rhs=P["cvec"][:, hf, :],
                                         start=False, stop=True)
                    xs = xt[:, cch, hf * 512:(hf + 1) * 512]
                    dst = osb[:, hf * 512:(hf + 1) * 512]
                    if (cch, hf) in RES_PAIR:
                        # Pool can't read PSUM: Act scales ud into SBUF,
                        # Pool adds the residual
                        uph = px.tile([128, 512], bf16, tag="uph", bufs=3,
                                      name=f"uph{t}_{cch}_{hf}")
                        nc.scalar.activation(uph, ud, AF.Identity,
                                             scale=1.0 / SP)
                        nc.gpsimd.tensor_tensor(out=dst, in0=uph, in1=xs,
                                                op=OP.add)
                    else:
                        nc.vector.scalar_tensor_tensor(
                            out=dst, in0=ud, scalar=1.0 / SP,
                            in1=xs, op0=OP.mult, op1=OP.add)
                nc.sync.dma_start(
                    out=out_d[row0 + cch * 128: row0 + (cch + 1) * 128, :],
                    in_=osb)

        ld = {}
        ee = {}
        a0 = {}
        ub = {}
        cmp_ = {}
        yc = {}
        # pipeline: load+stats@t a0@t+1 expand@t+2 comp/var@t+3 sel/ycT@t+4
        # proj/residual/out@t+5
        for s in range(ntile + 5):
            if s < ntile:
                ld[s] = stage_load(s)
            if s == 0:
                load_params_a()
            if s == 1:
                load_params_b()
            if s == 2:
                load_params_c()
            if 1 <= s <= ntile:
                t = s - 1
                a0[t] = stage_a0(t, ld[t][1], ee.pop(t))
            if 4 <= s <= ntile + 3:
                t = s - 4
                ybb, rrbb = cmp_.pop(t)
                yc[t] = stage_back_sel(t, ybb, rrbb)
            if 3 <= s <= ntile + 2:
                t = s - 3
                ybb2, sqy2 = stage_comp(t, ub.pop(t))
            if 2 <= s <= ntile + 1:
                t = s - 2
                ub[t] = stage_expand(t, a0.pop(t))
            if 5 <= s <= ntile + 4:
                t = s - 5
                stage_back_proj(t, ld.pop(t)[0], yc.pop(t))
            if 3 <= s <= ntile + 2:
                t = s - 3
                cmp_[t] = (ybb2, stage_var(t, sqy2))
            if s < ntile:
                t = s
                eebf_ = stage_stats_bn(t, ld[t][0])
                ee[t] = stage_stats_tp(t, eebf_)

    nc.compile()
    return nc


def _pack_params(anchors, ln_g, W1, b1, W2, b2, cg, cb, Wp, bp, gate):
    import ml_dtypes
    f32 = np.float32
    bf16 = ml_dtypes.bfloat16
    fp8 = ml_dtypes.float8_e4m3

    anchors = anchors.astype(f32)
    an = anchors / np.maximum(
        np.linalg.norm(anchors.astype(np.float64), axis=1, keepdims=True),
        1e-12).astype(f32)
    ag = an * ln_g[None, :].astype(f32)  # [A, D]
    a_c = ag - ag.mean(axis=1, keepdims=True)  # fold LN mean-centering

    # agt[p, s, i, 32r+m] = 2*a_c[m, 4p+s+512i], 4 row replicas
    agt = np.zeros((128, 4, 2, 128), f32)
    ww = np.arange(512)
    pidx, sidx = ww // 4, ww % 4
    for i in range(2):
        for r in range(4):
            agt[pidx, sidx, i, 32 * r:32 * r + A] = 2.0 * a_c.T[ww + 512 * i]

    # W1exp[m, k, e] = W1[k, j, e] iff m == j*C + k
    W1 = W1.astype(f32)
    w1exp = np.zeros((A, C, E2), f32)
    for m in range(A):
        j, k2 = m // C, m % C
        w1exp[m, k2, :] = W1[k2, j, :]
    biasu_np = w1exp.sum(axis=0) + b1.astype(f32)  # [C, E2]
    w1e = np.zeros((128, KD, 128), f32)
    for k in range(KD):
        r = k % 4
        w1e[32 * r:32 * r + A, k, :] = w1exp[:, k, :]
    biasu_sb = np.ascontiguousarray(biasu_np.T)  # [128, KD]

    W2 = W2.astype(f32)
    w2cent = W2 - W2.mean(axis=2, keepdims=True)
    w2c = np.transpose(w2cent, (1, 0, 2)).copy()  # [128, C, 64]
    b2c = b2.astype(f32) - b2.astype(f32).mean(axis=1, keepdims=True)

    b2f = np.zeros((128, 4), f32)
    for j in range(4):
        for p in range(128):
            kk = 2 * j + p // 64
            b2f[p, j] = b2c[kk, p % 64]

    vstl = np.zeros((128, 4, C), f32)
    for j in range(4):
        for p in range(128):
            vstl[p, j, 2 * j + p // 64] = 1.0 / DC

    sig = (1.0 / (1.0 + np.exp(-gate.astype(np.float64)))).astype(f32)
    wpfold = (cg.astype(f32).reshape(C * DC, 1) * Wp.astype(f32)) * sig[None, :]
    wpf = np.ascontiguousarray(
        (SP * wpfold).reshape(2, 2, 128, 2, 512).transpose(2, 0, 1, 3, 4))

    const = (cb.astype(f32).reshape(-1) @ Wp.astype(f32) + bp.astype(f32)) * sig
    use_const = bool(np.max(np.abs(const)) > 0)

    sel_np = np.zeros((C, 4, 128), f32)
    for j in range(4):
        sel_np[2 * j, j, 0:64] = 1.0
        sel_np[2 * j + 1, j, 64:128] = 1.0

    selc = np.zeros((4, 4, 128), f32)
    for c in range(4):
        selc[c, c, :] = 1.0

    params = dict(
        agt=agt.astype(fp8),
        w1e=w1e.astype(bf16),
        biasu=biasu_sb.astype(f32),
        w2c=w2c.astype(bf16),
        b2f=b2f.astype(f32),
        vstl=vstl.astype(bf16),
        sel=sel_np.astype(bf16),
        selc=selc.astype(bf16),
        ident=np.eye(128, dtype=f32).astype(bf16),
        idsp=(SP * np.eye(128, dtype=f32)).astype(bf16),
        wpf=wpf.astype(fp8),
    )
    if use_const:
        params["cvec"] = (SP * const).reshape(1, 2, 512).astype(bf16)
    return params, use_const


def _pack_x(xcore):
    """Pack one core's x [S, D] f32 into (token-major bf16, feature-major
    fp8-pair words viewed as bf16 [128, NTILE, 4, 512])."""
    import ml_dtypes
    bf16 = ml_dtypes.bfloat16
    fp8 = ml_dtypes.float8_e4m3
    xb = xcore.astype(bf16)
    x8 = (xcore / 8.0).astype(fp8)  # [S, D]
    # word[p, t, s, n] = (x8[512t+n, 4p+s], x8[512t+n, 4p+s+512])
    arr = x8.reshape(NTILE, TOK, 2, 128, 4).transpose(3, 0, 4, 1, 2)
    arr = np.ascontiguousarray(arr)  # [128, NTILE, 4, TOK, 2] fp8
    xT = arr.view(np.uint16).reshape(128, NTILE, 4, TOK).view(bf16)
    return xb, xT


def kernel(**inputs):
    x = np.asarray(inputs["x"], dtype=np.float32)
    ln_g = np.asarray(inputs["ln_g"], dtype=np.float32)
    ln_b = np.asarray(inputs["ln_b"], dtype=np.float32)

    fast = (np.allclose(ln_g, 1.0, atol=1e-12) and
            np.allclose(ln_b, 0.0, atol=1e-12))
    if not fast:
        return _np_reference(
            x, *[np.asarray(inputs[k], dtype=np.float32) for k in
                 ("anchors", "ln_g", "ln_b", "W1", "b1", "W2", "b2", "cg",
                  "cb", "Wp", "bp", "gate")])

    params, use_const = _pack_params(
        inputs["anchors"], ln_g, inputs["W1"], inputs["b1"], inputs["W2"],
        inputs["b2"], inputs["cg"], inputs["cb"], inputs["Wp"], inputs["bp"],
        inputs["gate"])

    nc = _build_program(S, use_const)

    from concourse.bass_utils import run_bass_kernel_spmd
    in_maps = []
    for b in range(NCORES):
        m = dict(params)
        xb, xT = _pack_x(np.ascontiguousarray(x[b]))
        m["xb"] = xb
        m["xT"] = xT
        in_maps.append(m)
    res = run_bass_kernel_spmd(nc, in_maps, core_ids=list(range(NCORES)))
    out = np.stack([np.asarray(res.results[b]["out"]).astype(np.float32)
                    for b in range(NCORES)], axis=0)
    return out.reshape(B, S, D)
